# revision 1
# baseline (speedup 1.0000x reference)
"""Trainium2 Bass kernel for nn_Discriminator_lstm (B=4096, T=32, E=H=300, VOCAB=10000).

Strategy (data-parallel over batch, 8 cores x 512 rows):
  Phase 1 (per core, replicated): G = embed_w @ W_ih^T  -> DRAM scratch [10000, 1280]
          (input projection folded into an embedding-table transform; the
           per-token input projection then becomes a row *gather* of G).
  Phase 2: 32-step LSTM recurrence.  Per step:
          - indirect-DMA gather xg = G[cap[:, t]] into SBUF
          - PE: inject xg into PSUM (identity matmul), accumulate
            gates += [h | 1] @ [W_hh^T ; b]  (bias rides an ones-row)
          - ACT: sigmoid on [i f o] (contiguous after gate reorder), tanh on g
          - DVE: c = f*c + i*g ; h = o*tanh(c); masked capture of h_last
          - PE transpose h -> hT for the next step's stationary operand
  Phase 3: logits = h_last @ Wc_norm^T + b  (weight-normed classifier, bias
          rides the ones-row again), DMA out.

Matmul inputs run as float32r (full PE rate at N>=256) or bf16 (MM_DT below).
"""

import os
import sys

import numpy as np

for _p in ("/opt/trn_rl_repo", "/root/.axon_site/_ro/trn_rl_repo"):
    if os.path.isdir(_p) and _p not in sys.path:
        sys.path.insert(0, _p)

import concourse.bass as bass
import concourse.bacc as bacc
import concourse.mybir as mybir
import concourse.tile as tile
from concourse.bass_utils import run_bass_kernel_spmd
from concourse.masks import make_identity

f32 = mybir.dt.float32
f32r = mybir.dt.float32r
bf16 = mybir.dt.bfloat16
i32 = mybir.dt.int32
u8 = mybir.dt.uint8

B, T, V, E, H = 4096, 32, 10000, 300, 300
NCORES = 8
BC = B // NCORES          # 512 batch rows per core
M = BC // 128             # 4 m-tiles
GATE_COLS = 1280          # 4*300 gates padded to 1280 (psum chunks all >= 256)
CHUNKS = [(0, 512), (512, 1024), (1024, 1280)]
E_SPLITS = [(0, 128), (128, 256), (256, 300)]       # k-tiles of the E contraction
K_SPLITS = [(0, 128), (128, 256), (256, 301)]       # k-tiles of the [h|1] contraction
H_SPLITS = [(0, 128), (128, 256), (256, 300)]       # h-dim splits for transposes
VTILES = (V + 127) // 128                            # 79

MM_DT = os.environ.get("KERNEL_MM_DT", "f32r")       # "f32r" | "bf16"
G_DT = os.environ.get("KERNEL_G_DT", MM_DT)          # "f32r" | "bf16"


def _raw(inst):
    return getattr(inst, "ins", inst)


def _set_row(nc, ap, row, value):
    """Set one partition row of an already-zeroed tile to `value`.

    Engine APs cannot start at arbitrary partitions, so write via
    affine_select over the whole tile: p == row ? value : in_.
    """
    nc.gpsimd.affine_select(
        out=ap,
        in_=ap,
        compare_op=mybir.AluOpType.not_equal,
        fill=value,
        base=-row,
        pattern=[[0, ap.shape[-1]]],
        channel_multiplier=1,
    )


def _mm(ap, mode):
    """Matmul operands are already declared in the matmul dtype."""
    return ap


def build_bass():
    mm_np_dt = f32r if MM_DT == "f32r" else bf16
    g_np_dt = f32r if G_DT == "f32r" else bf16

    nc = bacc.Bacc("TRN2", target_bir_lowering=False, debug=False, num_devices=NCORES)

    embT_d = nc.dram_tensor("embT", [E, V], mm_np_dt, kind="ExternalInput")
    wih_d = nc.dram_tensor("wih", [E, GATE_COLS], mm_np_dt, kind="ExternalInput")
    waug_d = nc.dram_tensor("waug", [H + 1, GATE_COLS], mm_np_dt, kind="ExternalInput")
    wc_d = nc.dram_tensor("wc", [H + 1, 2], mm_np_dt, kind="ExternalInput")
    ident_d = nc.dram_tensor("ident", [128, 128], mm_np_dt, kind="ExternalInput")
    identg_d = nc.dram_tensor("identg", [128, 128], g_np_dt, kind="ExternalInput")
    hT_init_d = nc.dram_tensor("hT_init", [H + 1, BC], mm_np_dt, kind="ExternalInput")
    idx_d = nc.dram_tensor("idx", [128, T, M], i32, kind="ExternalInput")
    mask_d = nc.dram_tensor("mask", [128, T, M], u8, kind="ExternalInput")
    G_d = nc.dram_tensor("G", [V, GATE_COLS], g_np_dt, kind="Internal")
    out_d = nc.dram_tensor("out", [BC, 2], f32, kind="ExternalOutput")

    with tile.TileContext(nc, num_cores=NCORES) as tc:
        with (
            tc.tile_pool(name="const", bufs=1) as cpool,
            tc.tile_pool(name="state", bufs=1) as spool,
            tc.tile_pool(name="psum", bufs=2, space="PSUM") as ppool,
        ):
            # ---------- constants ----------
            ident = cpool.tile([128, 128], mm_np_dt, tag="ident")
            nc.sync.dma_start(out=ident[:, :], in_=ident_d[:, :])
            identg = ident
            if g_np_dt != mm_np_dt:
                identg = cpool.tile([128, 128], g_np_dt, tag="identg")
                nc.sync.dma_start(out=identg[:, :], in_=identg_d[:, :])

            waug_sb = []
            for k, (d0, d1) in enumerate(K_SPLITS):
                t_ = cpool.tile([d1 - d0, GATE_COLS], mm_np_dt, tag=f"waug{k}")
                nc.sync.dma_start(out=t_[:, :], in_=waug_d[d0:d1, :])
                waug_sb.append(t_)
            wc_sb = []
            for k, (d0, d1) in enumerate(K_SPLITS):
                t_ = cpool.tile([d1 - d0, 2], mm_np_dt, tag=f"wc{k}")
                nc.sync.dma_start(out=t_[:, :], in_=wc_d[d0:d1, :])
                wc_sb.append(t_)
            idx_sb = cpool.tile([128, T, M], i32, tag="idx")
            nc.sync.dma_start(out=idx_sb[:, :, :], in_=idx_d[:, :, :])
            mask_sb = cpool.tile([128, T, M], u8, tag="mask")
            nc.sync.dma_start(out=mask_sb[:, :, :], in_=mask_d[:, :, :])

            # ---------- state ----------
            hT = []
            for k, (d0, d1) in enumerate(K_SPLITS):
                t_ = spool.tile([d1 - d0, BC], mm_np_dt, tag=f"hT{k}")
                nc.sync.dma_start(out=t_[:, :], in_=hT_init_d[d0:d1, :])
                hT.append(t_)
            c_sb = spool.tile([128, M, H], f32, tag="c")
            nc.gpsimd.memset(c_sb[:, :, :], 0.0)
            # h_last needs no init: every row is written at exactly one step
            h_last = spool.tile([128, M, H], f32 if MM_DT == "f32r" else mm_np_dt, tag="hlast")
            hlT = []
            for k, (d0, d1) in enumerate(K_SPLITS):
                t_ = spool.tile([d1 - d0, BC], mm_np_dt, tag=f"hlT{k}")
                nc.sync.dma_start(out=t_[:, :], in_=hT_init_d[d0:d1, :])
                hlT.append(t_)

            # ---------- phase 1: G = embT^T @ wih ----------
            g_stores = []
            with tc.tile_pool(name="gphase", bufs=1) as gpool, \
                 tc.tile_pool(name="gsbp", bufs=3) as gsbp:
                wih_sb = []
                for k, (d0, d1) in enumerate(E_SPLITS):
                    t_ = gpool.tile([d1 - d0, GATE_COLS], mm_np_dt, tag=f"wih{k}")
                    nc.sync.dma_start(out=t_[:, :], in_=wih_d[d0:d1, :])
                    wih_sb.append(t_)
                embT_sb = []
                for k, (d0, d1) in enumerate(E_SPLITS):
                    t_ = gpool.tile([d1 - d0, V], mm_np_dt, tag=f"emb{k}")
                    for q in range(4):
                        eng = nc.sync if q % 2 == 0 else nc.scalar
                        eng.dma_start(out=t_[:, q * (V // 4):(q + 1) * (V // 4)],
                                      in_=embT_d[d0:d1, q * (V // 4):(q + 1) * (V // 4)])
                    embT_sb.append(t_)

                for v in range(VTILES):
                    rows = min(128, V - v * 128)
                    gps = ppool.tile([128, GATE_COLS], f32, tag="gates")
                    for k, (d0, d1) in enumerate(E_SPLITS):
                        for (c0, c1) in CHUNKS:
                            nc.tensor.matmul(
                                gps[0:rows, c0:c1],
                                lhsT=_mm(embT_sb[k][:, v * 128:v * 128 + rows], MM_DT),
                                rhs=_mm(wih_sb[k][:, c0:c1], MM_DT),
                                start=(k == 0),
                                stop=(k == 2),
                            )
                    gsb = gsbp.tile([128, GATE_COLS], g_np_dt, tag="gsb")
                    if v % 2 == 0:
                        nc.vector.tensor_copy(gsb[0:rows, :], gps[0:rows, :])
                    else:
                        nc.scalar.copy(gsb[0:rows, :], gps[0:rows, :])
                    eng = nc.sync if v % 2 == 0 else nc.scalar
                    st = eng.dma_start(
                        out=G_d[v * 128:v * 128 + rows, :], in_=gsb[0:rows, :]
                    )
                    g_stores.append(st)

            g_done = nc.gpsimd.nop()
            for st in g_stores:
                tile.add_dep_helper(_raw(g_done), _raw(st), reason="G stored")

            # ---------- phase 2: recurrence ----------
            with tc.tile_pool(name="work", bufs=2) as wpool:
                for t in range(T):
                    xg = wpool.tile([128, M, GATE_COLS], g_np_dt, tag="xg")
                    for m in range(M):
                        gather = nc.gpsimd.indirect_dma_start(
                            out=xg[:, m, :],
                            out_offset=None,
                            in_=G_d[:, :],
                            in_offset=bass.IndirectOffsetOnAxis(
                                ap=idx_sb[:, t, m:m + 1], axis=0),
                        )
                        tile.add_dep_helper(_raw(gather), _raw(g_done),
                                            reason="gather after G")

                    ifo = wpool.tile([128, M, 900], f32, tag="ifo")
                    gt = wpool.tile([128, M, 300], f32, tag="gt")
                    tch = wpool.tile([128, M, H], f32, tag="tch")
                    h_sb = wpool.tile([128, M, H],
                                      f32 if MM_DT == "f32r" else mm_np_dt, tag="h")
                    t1 = wpool.tile([128, M, H], f32, tag="t1")
                    ident_tr = (ident[:, :].bitcast(f32) if MM_DT == "f32r"
                                else ident[:, :])
                    for m in range(M):
                        gps = ppool.tile([128, GATE_COLS], f32, tag="gates")
                        # inject xg (clears + seeds accumulation per bank)
                        for (c0, c1) in CHUNKS:
                            nc.tensor.matmul(
                                gps[:, c0:c1],
                                lhsT=_mm(identg[:, :], G_DT),
                                rhs=_mm(xg[:, m, c0:c1], G_DT),
                                start=True,
                                stop=False,
                            )
                        # gates += [h | 1] @ waug
                        for k, (d0, d1) in enumerate(K_SPLITS):
                            lhs = hT[k][:, m * 128:(m + 1) * 128]
                            for (c0, c1) in CHUNKS:
                                nc.tensor.matmul(
                                    gps[:, c0:c1],
                                    lhsT=_mm(lhs, MM_DT),
                                    rhs=_mm(waug_sb[k][:, c0:c1], MM_DT),
                                    start=False,
                                    stop=(k == 2),
                                )
                        nc.scalar.activation(
                            ifo[:, m, :], gps[:, 0:900],
                            mybir.ActivationFunctionType.Sigmoid,
                        )
                        nc.scalar.activation(
                            gt[:, m, :], gps[:, 900:1200],
                            mybir.ActivationFunctionType.Tanh,
                        )
                        # per-m state update: c = f*c + i*g ; h = o*tanh(c)
                        nc.vector.tensor_tensor(
                            out=t1[:, m, :], in0=ifo[:, m, 300:600],
                            in1=c_sb[:, m, :], op=mybir.AluOpType.mult,
                        )
                        nc.vector.tensor_tensor(
                            out=c_sb[:, m, :], in0=ifo[:, m, 0:300],
                            in1=gt[:, m, :], op=mybir.AluOpType.mult,
                        )
                        nc.vector.tensor_tensor(
                            out=c_sb[:, m, :], in0=c_sb[:, m, :],
                            in1=t1[:, m, :], op=mybir.AluOpType.add,
                        )
                        nc.scalar.activation(
                            tch[:, m, :], c_sb[:, m, :],
                            mybir.ActivationFunctionType.Tanh,
                        )
                        nc.vector.tensor_tensor(
                            out=h_sb[:, m, :], in0=ifo[:, m, 600:900],
                            in1=tch[:, m, :], op=mybir.AluOpType.mult,
                        )
                        nc.vector.copy_predicated(
                            out=h_last[:, m, :],
                            mask=mask_sb[:, t, m:m + 1].to_broadcast([128, H]),
                            data=h_sb[:, m, :],
                        )
                        # h -> hT for the next step (per-m transpose + drain)
                        trp = ppool.tile([128, 384], f32 if MM_DT == "f32r"
                                         else mm_np_dt, tag="tr")
                        for k, (d0, d1) in enumerate(H_SPLITS):
                            dk = d1 - d0
                            nc.tensor.transpose(
                                out=trp[0:dk, k * 128:k * 128 + 128],
                                in_=h_sb[:, m, d0:d1],
                                identity=ident_tr,
                            )
                        for k, (d0, d1) in enumerate(H_SPLITS):
                            dk = d1 - d0
                            dst = hT[k][0:dk, m * 128:(m + 1) * 128]
                            srcp = trp[0:dk, k * 128:k * 128 + 128]
                            if (m + k) % 2 == 0:
                                nc.vector.tensor_copy(dst, srcp)
                            else:
                                nc.scalar.copy(dst, srcp)

                # ---------- phase 3: logits ----------
                trps = []
                for k in range(3):
                    trp = ppool.tile([128, 512], f32 if MM_DT == "f32r" else mm_np_dt, tag="tr")
                    trps.append(trp)
                for m in range(M):
                    for k, (d0, d1) in enumerate(H_SPLITS):
                        dk = d1 - d0
                        nc.tensor.transpose(
                            out=_mm(trps[k][0:dk, m * 128:(m + 1) * 128], MM_DT),
                            in_=_mm(h_last[:, m, d0:d1], MM_DT),
                            identity=ident[:, :].bitcast(f32) if MM_DT == "f32r" else ident[:, :],
                        )
                for k, (d0, d1) in enumerate(H_SPLITS):
                    dk = d1 - d0
                    nc.vector.tensor_copy(hlT[k][0:dk, :], trps[k][0:dk, :])

                lsb = wpool.tile([128, M, 2], f32, tag="lsb")
                for m in range(M):
                    lp = ppool.tile([128, 2], f32, tag="tr")
                    for k, (d0, d1) in enumerate(K_SPLITS):
                        nc.tensor.matmul(
                            lp[:, :],
                            lhsT=_mm(hlT[k][:, m * 128:(m + 1) * 128], MM_DT),
                            rhs=_mm(wc_sb[k][:, :], MM_DT),
                            start=(k == 0),
                            stop=(k == 2),
                        )
                    nc.vector.tensor_copy(lsb[:, m, :], lp[:, :])
                nc.sync.dma_start(
                    out=out_d[:, :].rearrange("(m p) c -> p m c", p=128), in_=lsb[:, :, :]
                )

    nc.compile()
    return nc


_NC_CACHE = {}
LAST_RESULT = None


def _host_prep(inputs):
    mm_np = np.float32
    g_np = np.float32
    if MM_DT == "bf16" or G_DT == "bf16":
        import ml_dtypes
        if MM_DT == "bf16":
            mm_np = ml_dtypes.bfloat16
        if G_DT == "bf16":
            g_np = ml_dtypes.bfloat16
    del g_np  # G dtype handled on device (psum -> gsb copy casts)

    cap = np.asarray(inputs["cap"]).astype(np.int32)
    cap_len = np.asarray(inputs["cap_len"]).astype(np.int32)
    embed_w = np.asarray(inputs["embed_w"], dtype=np.float32)
    W_ih = np.asarray(inputs["W_ih"], dtype=np.float32)
    W_hh = np.asarray(inputs["W_hh"], dtype=np.float32)
    b = (np.asarray(inputs["b_ih"], dtype=np.float32)
         + np.asarray(inputs["b_hh"], dtype=np.float32))
    cls_v = np.asarray(inputs["cls_v"], dtype=np.float32)
    cls_g = np.asarray(inputs["cls_g"], dtype=np.float32)
    cls_b = np.asarray(inputs["cls_b"], dtype=np.float32)

    perm = np.concatenate([np.arange(0, 300), np.arange(300, 600),
                           np.arange(900, 1200), np.arange(600, 900)])  # i f o g
    wih_t = np.zeros((E, GATE_COLS), np.float32)
    wih_t[:, :1200] = W_ih[perm].T
    waug = np.zeros((H + 1, GATE_COLS), np.float32)
    waug[:H, :1200] = W_hh[perm].T
    waug[H, :1200] = b[perm]
    Wc = cls_g * cls_v / np.linalg.norm(cls_v, axis=1, keepdims=True)  # [2, 300]
    wc = np.zeros((H + 1, 2), np.float32)
    wc[:H] = Wc.T
    wc[H] = cls_b
    embT = np.ascontiguousarray(embed_w.T)  # [300, 10000]

    eye = np.eye(128, dtype=np.float32)
    hT_init = np.zeros((H + 1, BC), np.float32)
    hT_init[H] = 1.0
    shared = {
        "ident": eye.astype(mm_np),
        "identg": eye.astype(mm_np if G_DT == MM_DT else (
            __import__("ml_dtypes").bfloat16 if G_DT == "bf16" else np.float32)),
        "hT_init": hT_init.astype(mm_np),
        "embT": embT.astype(mm_np),
        "wih": wih_t.astype(mm_np),
        "waug": waug.astype(mm_np),
        "wc": wc.astype(mm_np),
    }
    in_maps = []
    for core in range(NCORES):
        capc = cap[core * BC:(core + 1) * BC]          # [512, 32]
        lenc = cap_len[core * BC:(core + 1) * BC]      # [512]
        idx = np.ascontiguousarray(
            capc.reshape(M, 128, T).transpose(1, 2, 0)).astype(np.int32)  # [128,T,M]
        lm = lenc.reshape(M, 128).T                    # [128, M]
        mask = (lm[:, None, :] - 1 == np.arange(T)[None, :, None]).astype(np.uint8)
        in_maps.append(dict(shared, idx=idx, mask=np.ascontiguousarray(mask)))
    return in_maps


def kernel(**inputs) -> np.ndarray:
    global LAST_RESULT
    key = (MM_DT, G_DT)
    if key not in _NC_CACHE:
        _NC_CACHE[key] = build_bass()
    nc = _NC_CACHE[key]
    in_maps = _host_prep(inputs)
    trace = bool(int(os.environ.get("KERNEL_TRACE", "0")))
    res = run_bass_kernel_spmd(nc, in_maps, core_ids=list(range(NCORES)), trace=trace)
    LAST_RESULT = res
    out = np.concatenate([r["out"] for r in res.results], axis=0)
    return out.astype(np.float32)



# revision 12
# speedup vs baseline: 1.3806x; 1.3806x over previous
"""Trainium2 Bass kernel for nn_Discriminator_lstm (B=4096, T=32, E=H=300, VOCAB=10000).

Strategy (data-parallel over batch, 8 cores x 512 rows):
  Host: globally sort rows by cap_len, deal ranks round-robin to cores
        (every core gets the same length distribution), sorted ascending
        within each core.  m-tile m then has max length steps[m] (~8/16/24/32),
        and the recurrence runs only steps[m] steps for tile m.
  Phase 1 (per core): G = [embT;1] ^T @ [wih;b] in bf16 -> DRAM scratch
        [10000, 1200], scaled by S=512 (bias folded in via ones-row).
  Phase 2: per live (t, m):
        - indirect-DMA gather xg = G[cap[:, t]] (bf16) into SBUF
        - PE: inject xg into PSUM (identity matmul, bf16), then accumulate
          gates += hT8^T @ waug2 as TWO fp8e4m3 DoubleRow matmuls
          (k-slots [0:2] and [2:4]; h scaled by s_h=16, W_hh by s_w=32,
          psum scale S = s_h*s_w = 512)
        - ACT: sigmoid/tanh with scale=1/S -> bf16
        - DVE: c = f*c + i*g (f32); h_bf = o*tanh(c); masked h_last capture
        - Pool: h8 = (o*s_h)*tanh(c) -> fp8 for the next step's matmul
        - PE: transpose h8 (fp8, 1 cyc/row) -> one merged DVE copy into hT8
  Phase 3: logits = h_last @ Wc^T + cls_b in bf16.
"""

import os
import sys

import numpy as np

for _p in ("/opt/trn_rl_repo", "/root/.axon_site/_ro/trn_rl_repo"):
    if os.path.isdir(_p) and _p not in sys.path:
        sys.path.insert(0, _p)

import ml_dtypes

import concourse.bass as bass
import concourse.bacc as bacc
import concourse.mybir as mybir
import concourse.tile as tile

f32 = mybir.dt.float32
bf16 = mybir.dt.bfloat16
fp8 = mybir.dt.float8e4
i32 = mybir.dt.int32
u8 = mybir.dt.uint8

np_bf16 = ml_dtypes.bfloat16
np_fp8 = ml_dtypes.float8_e4m3

B, T, V, E, H = 4096, 32, 10000, 300, 300
NCORES = 8
BC = B // NCORES          # 512 batch rows per core
M = BC // 128             # 4 m-tiles
GC = 1200                 # 4*300 gate columns
CHUNKS = [(0, 512), (512, 1024), (1024, 1200)]
E_SPLITS = [(0, 128), (128, 256), (256, 301)]   # k-tiles of the [emb|1] contraction
H_SPLITS = [(0, 128), (128, 256), (256, 300)]   # h-dim splits for transposes/classifier
VTILES = (V + 127) // 128                        # 79

S_H = 16.0                # fp8 h scale
S_W = 32.0                # fp8 W_hh scale
S = S_H * S_W             # psum gate scale


def _raw(inst):
    return getattr(inst, "ins", inst)


def build_bass(steps):
    nc = bacc.Bacc("TRN2", target_bir_lowering=False, debug=False, num_devices=NCORES)

    embT_d = nc.dram_tensor("embT", [E + 1, V], bf16, kind="ExternalInput")
    wih_d = nc.dram_tensor("wih", [E + 1, GC], bf16, kind="ExternalInput")
    waug2_d = nc.dram_tensor("waug2", [128, 4, GC], fp8, kind="ExternalInput")
    wc_d = nc.dram_tensor("wc", [H, 2], bf16, kind="ExternalInput")
    clsb_d = nc.dram_tensor("clsb", [128, 2], f32, kind="ExternalInput")
    identb_d = nc.dram_tensor("identb", [128, 128], bf16, kind="ExternalInput")
    ident8_d = nc.dram_tensor("ident8", [128, 128], fp8, kind="ExternalInput")
    idx_d = nc.dram_tensor("idx", [128, T, M], i32, kind="ExternalInput")
    mask_d = nc.dram_tensor("mask", [128, T, M], u8, kind="ExternalInput")
    G_d = nc.dram_tensor("G", [V, GC], bf16, kind="Internal")
    out_d = nc.dram_tensor("out", [BC, 2], f32, kind="ExternalOutput")

    with tile.TileContext(nc, num_cores=NCORES) as tc:
        with (
            tc.tile_pool(name="const", bufs=1) as cpool,
            tc.tile_pool(name="state", bufs=1) as spool,
            tc.tile_pool(name="psum", bufs=2, space="PSUM") as ppool,
        ):
            # ---------- constants ----------
            identb = cpool.tile([128, 128], bf16, tag="identb")
            nc.sync.dma_start(out=identb[:, :], in_=identb_d[:, :])
            ident8 = cpool.tile([128, 128], fp8, tag="ident8")
            nc.sync.dma_start(out=ident8[:, :], in_=ident8_d[:, :])
            waug2 = cpool.tile([128, 4, GC], fp8, tag="waug2")
            nc.sync.dma_start(out=waug2[:, :, :], in_=waug2_d[:, :, :])
            wc_sb = []
            for k, (d0, d1) in enumerate(H_SPLITS):
                t_ = cpool.tile([d1 - d0, 2], bf16, tag=f"wc{k}")
                nc.sync.dma_start(out=t_[:, :], in_=wc_d[d0:d1, :])
                wc_sb.append(t_)
            clsb = cpool.tile([128, 2], f32, tag="clsb")
            nc.sync.dma_start(out=clsb[:, :], in_=clsb_d[:, :])
            idx_sb = cpool.tile([128, T, M], i32, tag="idx")
            nc.sync.dma_start(out=idx_sb[:, :, :], in_=idx_d[:, :, :])
            mask_sb = cpool.tile([128, T, M], u8, tag="mask")
            nc.sync.dma_start(out=mask_sb[:, :, :], in_=mask_d[:, :, :])

            # ---------- state ----------
            # hT8 slots: [0]=h rows 0:128, [1]=128:256, [2]=256:300 (+garbage,
            # nulled by zero rows in waug2), [3]=zeros in waug2 -> don't care.
            hT8 = spool.tile([128, 4, BC], fp8, tag="hT8")
            nc.gpsimd.memset(hT8[:, :, :], 0.0)
            c_sb = spool.tile([128, M, H], f32, tag="c")
            nc.gpsimd.memset(c_sb[:, :, :], 0.0)
            h_last = spool.tile([128, M, H], bf16, tag="hlast")
            hlT = []
            for k, (d0, d1) in enumerate(H_SPLITS):
                t_ = spool.tile([d1 - d0, BC], bf16, tag=f"hlT{k}")
                hlT.append(t_)

            # ---------- phase 1: G = [embT;1]^T @ [wih;b], scaled by S ----------
            g_stores = []
            with tc.tile_pool(name="gphase", bufs=1) as gpool, \
                 tc.tile_pool(name="gsbp", bufs=3) as gsbp:
                wih_sb = []
                for k, (d0, d1) in enumerate(E_SPLITS):
                    t_ = gpool.tile([d1 - d0, GC], bf16, tag=f"wih{k}")
                    nc.sync.dma_start(out=t_[:, :], in_=wih_d[d0:d1, :])
                    wih_sb.append(t_)
                embT_sb = []
                for k, (d0, d1) in enumerate(E_SPLITS):
                    t_ = gpool.tile([d1 - d0, V], bf16, tag=f"emb{k}")
                    for q in range(4):
                        nc.sync.dma_start(
                            out=t_[:, q * (V // 4):(q + 1) * (V // 4)],
                            in_=embT_d[d0:d1, q * (V // 4):(q + 1) * (V // 4)])
                    embT_sb.append(t_)

                for v in range(VTILES):
                    rows = min(128, V - v * 128)
                    gps = ppool.tile([128, GC], f32, tag="gates")
                    for k, (d0, d1) in enumerate(E_SPLITS):
                        for (c0, c1) in CHUNKS:
                            nc.tensor.matmul(
                                gps[0:rows, c0:c1],
                                lhsT=embT_sb[k][:, v * 128:v * 128 + rows],
                                rhs=wih_sb[k][:, c0:c1],
                                start=(k == 0),
                                stop=(k == 2),
                            )
                    gsb = gsbp.tile([128, GC], bf16, tag="gsb")
                    if v % 2 == 0:
                        nc.scalar.activation(
                            gsb[0:rows, :], gps[0:rows, :],
                            mybir.ActivationFunctionType.Copy, scale=S)
                    else:
                        nc.vector.tensor_scalar(
                            out=gsb[0:rows, :], in0=gps[0:rows, :],
                            scalar1=S, scalar2=None, op0=mybir.AluOpType.mult)
                    st = nc.sync.dma_start(
                        out=G_d[v * 128:v * 128 + rows, :], in_=gsb[0:rows, :]
                    )
                    g_stores.append(st)

            g_done = nc.gpsimd.nop()
            for st in g_stores:
                tile.add_dep_helper(_raw(g_done), _raw(st), reason="G stored")

            # ---------- phase 2: recurrence ----------
            with tc.tile_pool(name="work", bufs=2) as wpool:
                for t in range(T):
                    ms = [m for m in range(M) if steps[m] > t]
                    xg = wpool.tile([128, M, GC], bf16, tag="xg")
                    for m in ms:
                        gather = nc.gpsimd.indirect_dma_start(
                            out=xg[:, m, :],
                            out_offset=None,
                            in_=G_d[:, :],
                            in_offset=bass.IndirectOffsetOnAxis(
                                ap=idx_sb[:, t, m:m + 1], axis=0),
                        )
                        tile.add_dep_helper(_raw(gather), _raw(g_done),
                                            reason="gather after G")

                    ifo = wpool.tile([128, M, 900], bf16, tag="ifo")
                    gt = wpool.tile([128, M, 300], bf16, tag="gt")
                    tch = wpool.tile([128, M, H], bf16, tag="tch")
                    hbf = wpool.tile([128, M, H], bf16, tag="hbf")
                    t1 = wpool.tile([128, M, H], f32, tag="t1")
                    for m in ms:
                        gps = ppool.tile([128, GC], f32, tag="gates")
                        # inject xg (clears + seeds accumulation per chunk)
                        for (c0, c1) in CHUNKS:
                            nc.tensor.matmul(
                                gps[:, c0:c1],
                                lhsT=identb[:, :],
                                rhs=xg[:, m, c0:c1],
                                start=True,
                                stop=(t == 0),
                            )
                        if t > 0:
                            # gates += hT8^T @ waug2, fp8 DoubleRow (2 k-slot pairs)
                            for j, (s0, s1) in enumerate(((0, 2), (2, 4))):
                                for (c0, c1) in CHUNKS:
                                    nc.tensor.matmul(
                                        gps[:, c0:c1],
                                        lhsT=hT8[:, s0:s1, m * 128:(m + 1) * 128],
                                        rhs=waug2[:, s0:s1, c0:c1],
                                        start=False,
                                        stop=(j == 1),
                                        perf_mode=mybir.MatmulPerfMode.DoubleRow,
                                    )
                        nc.scalar.activation(
                            ifo[:, m, :], gps[:, 0:900],
                            mybir.ActivationFunctionType.Sigmoid, scale=1.0 / S)
                        nc.scalar.activation(
                            gt[:, m, :], gps[:, 900:1200],
                            mybir.ActivationFunctionType.Tanh, scale=1.0 / S)
                        # c = f*c + i*g ; h = o*tanh(c)
                        nc.vector.tensor_tensor(
                            out=t1[:, m, :], in0=ifo[:, m, 300:600],
                            in1=c_sb[:, m, :], op=mybir.AluOpType.mult)
                        nc.vector.tensor_tensor(
                            out=c_sb[:, m, :], in0=ifo[:, m, 0:300],
                            in1=gt[:, m, :], op=mybir.AluOpType.mult)
                        nc.vector.tensor_tensor(
                            out=c_sb[:, m, :], in0=c_sb[:, m, :],
                            in1=t1[:, m, :], op=mybir.AluOpType.add)
                        nc.scalar.activation(
                            tch[:, m, :], c_sb[:, m, :],
                            mybir.ActivationFunctionType.Tanh)
                        nc.vector.tensor_tensor(
                            out=hbf[:, m, :], in0=ifo[:, m, 600:900],
                            in1=tch[:, m, :], op=mybir.AluOpType.mult)
                        nc.vector.copy_predicated(
                            out=h_last[:, m, :],
                            mask=mask_sb[:, t, m:m + 1].to_broadcast([128, H]),
                            data=hbf[:, m, :])
                        if t + 1 < steps[m]:
                            trp = ppool.tile([128, 3, 128], bf16, tag="tr")
                            for k, (d0, d1) in enumerate(H_SPLITS):
                                dk = d1 - d0
                                nc.tensor.transpose(
                                    out=trp[0:dk, k, :],
                                    in_=hbf[:, m, d0:d1],
                                    identity=identb[:, :])
                            # scaled fp8 convert during the psum->sbuf drain
                            nc.vector.tensor_scalar(
                                out=hT8[:, 0:2, m * 128:(m + 1) * 128],
                                in0=trp[:, 0:2, :], scalar1=S_H, scalar2=None,
                                op0=mybir.AluOpType.mult)
                            nc.scalar.activation(
                                hT8[0:44, 2, m * 128:(m + 1) * 128],
                                trp[0:44, 2, :],
                                mybir.ActivationFunctionType.Copy, scale=S_H)

                # ---------- phase 3: logits ----------
                trps = []
                for k in range(3):
                    trp = ppool.tile([128, 512], bf16, tag="gates")
                    trps.append(trp)
                for m in range(M):
                    for k, (d0, d1) in enumerate(H_SPLITS):
                        dk = d1 - d0
                        nc.tensor.transpose(
                            out=trps[k][0:dk, m * 128:(m + 1) * 128],
                            in_=h_last[:, m, d0:d1],
                            identity=identb[:, :])
                for k, (d0, d1) in enumerate(H_SPLITS):
                    dk = d1 - d0
                    nc.vector.tensor_copy(hlT[k][0:dk, :], trps[k][0:dk, :])

                lsb = wpool.tile([128, M, 2], f32, tag="lsb")
                for m in range(M):
                    lp = ppool.tile([128, 2], f32, tag="tr")
                    for k, (d0, d1) in enumerate(H_SPLITS):
                        nc.tensor.matmul(
                            lp[:, :],
                            lhsT=hlT[k][:, m * 128:(m + 1) * 128],
                            rhs=wc_sb[k][:, :],
                            start=(k == 0),
                            stop=(k == 2),
                        )
                    nc.vector.scalar_tensor_tensor(
                        out=lsb[:, m, :], in0=lp[:, :], scalar=1.0,
                        in1=clsb[:, :],
                        op0=mybir.AluOpType.mult, op1=mybir.AluOpType.add)
                nc.sync.dma_start(
                    out=out_d[:, :].rearrange("(m p) c -> p m c", p=128),
                    in_=lsb[:, :, :])

    nc.compile()
    return nc


_NC_CACHE = {}
LAST_RESULT = None


def _host_prep(inputs):
    cap = np.asarray(inputs["cap"]).astype(np.int64)
    cap_len = np.asarray(inputs["cap_len"]).astype(np.int64)
    embed_w = np.asarray(inputs["embed_w"], dtype=np.float32)
    W_ih = np.asarray(inputs["W_ih"], dtype=np.float32)
    W_hh = np.asarray(inputs["W_hh"], dtype=np.float32)
    b = (np.asarray(inputs["b_ih"], dtype=np.float32)
         + np.asarray(inputs["b_hh"], dtype=np.float32))
    cls_v = np.asarray(inputs["cls_v"], dtype=np.float32)
    cls_g = np.asarray(inputs["cls_g"], dtype=np.float32)
    cls_b = np.asarray(inputs["cls_b"], dtype=np.float32)

    # gate order [i f o g]
    perm = np.concatenate([np.arange(0, 300), np.arange(300, 600),
                           np.arange(900, 1200), np.arange(600, 900)])
    wih_aug = np.zeros((E + 1, GC), np.float32)
    wih_aug[:E] = W_ih[perm].T
    wih_aug[E] = b[perm]
    embT_aug = np.ones((E + 1, V), np.float32)
    embT_aug[:E] = embed_w.T

    Wp = W_hh[perm].T * S_W                          # [300, 1200], scaled
    waug2 = np.zeros((128, 4, GC), np.float32)
    waug2[:, 0, :] = Wp[0:128]
    waug2[:, 1, :] = Wp[128:256]
    waug2[0:44, 2, :] = Wp[256:300]

    Wc = cls_g * cls_v / np.linalg.norm(cls_v, axis=1, keepdims=True)  # [2, 300]

    # global sort by length; deal round-robin to cores
    order = np.argsort(cap_len, kind="stable")
    steps = []
    for m in range(M):
        mx = 0
        for c in range(NCORES):
            sel = order[c::NCORES]
            mx = max(mx, int(cap_len[sel[m * 128:(m + 1) * 128]].max()))
        steps.append(mx)
    steps = tuple(steps)

    shared = {
        "identb": np.eye(128, dtype=np.float32).astype(np_bf16),
        "ident8": np.eye(128, dtype=np.float32).astype(np_fp8),
        "embT": embT_aug.astype(np_bf16),
        "wih": wih_aug.astype(np_bf16),
        "waug2": waug2.astype(np_fp8),
        "wc": Wc.T.astype(np_bf16),
        "clsb": np.tile(cls_b.reshape(1, 2), (128, 1)).astype(np.float32),
    }
    in_maps = []
    for core in range(NCORES):
        sel = order[core::NCORES]
        capc = cap[sel]                                # [512, 32]
        lenc = cap_len[sel]                            # [512]
        idx = np.ascontiguousarray(
            capc.reshape(M, 128, T).transpose(1, 2, 0)).astype(np.int32)
        lm = lenc.reshape(M, 128).T                    # [128, M]
        mask = (lm[:, None, :] - 1 == np.arange(T)[None, :, None]).astype(np.uint8)
        in_maps.append(dict(shared, idx=idx, mask=np.ascontiguousarray(mask)))
    return in_maps, order, steps


def kernel(**inputs) -> np.ndarray:
    global LAST_RESULT
    from concourse.bass_utils import run_bass_kernel_spmd
    in_maps, order, steps = _host_prep(inputs)
    if steps not in _NC_CACHE:
        _NC_CACHE[steps] = build_bass(steps)
    nc = _NC_CACHE[steps]
    trace = bool(int(os.environ.get("KERNEL_TRACE", "0")))
    res = run_bass_kernel_spmd(nc, in_maps, core_ids=list(range(NCORES)), trace=trace)
    LAST_RESULT = res
    out = np.empty((B, 2), np.float32)
    for core in range(NCORES):
        out[order[core::NCORES]] = res.results[core]["out"].astype(np.float32)
    return out


# revision 17
# speedup vs baseline: 1.5426x; 1.1174x over previous
"""Trainium2 Bass kernel for nn_Discriminator_lstm (B=4096, T=32, E=H=300, VOCAB=10000).

Strategy (data-parallel over batch, 8 cores x 512 rows):
  Host: globally sort rows by cap_len, deal ranks round-robin to cores
        (every core gets the same length distribution), sorted ascending
        within each core.  m-tile m then has max length steps[m] (~8/16/24/32),
        and the recurrence runs only steps[m] steps for tile m.
  Phase 1 (per core): G = [embT;1] ^T @ [wih;b] in bf16 -> DRAM scratch
        [10000, 1200], scaled by S=512 (bias folded in via ones-row).
  Phase 2: per live (t, m):
        - indirect-DMA gather xg = G[cap[:, t]] (bf16) into SBUF
        - PE: inject xg into PSUM (identity matmul, bf16), then accumulate
          gates += hT8^T @ waug2 as TWO fp8e4m3 DoubleRow matmuls
          (k-slots [0:2] and [2:4]; h scaled by s_h=16, W_hh by s_w=32,
          psum scale S = s_h*s_w = 512)
        - ACT: sigmoid/tanh with scale=1/S -> bf16
        - DVE: c = f*c + i*g (f32); h_bf = o*tanh(c); masked h_last capture
        - Pool: h8 = (o*s_h)*tanh(c) -> fp8 for the next step's matmul
        - PE: transpose h8 (fp8, 1 cyc/row) -> one merged DVE copy into hT8
  Phase 3: logits = h_last @ Wc^T + cls_b in bf16.
"""

import os
import sys

import numpy as np

for _p in ("/opt/trn_rl_repo", "/root/.axon_site/_ro/trn_rl_repo"):
    if os.path.isdir(_p) and _p not in sys.path:
        sys.path.insert(0, _p)

import ml_dtypes

import concourse.bass as bass
import concourse.bacc as bacc
import concourse.mybir as mybir
import concourse.tile as tile

f32 = mybir.dt.float32
bf16 = mybir.dt.bfloat16
fp8 = mybir.dt.float8e4
i32 = mybir.dt.int32
u8 = mybir.dt.uint8

np_bf16 = ml_dtypes.bfloat16
np_fp8 = ml_dtypes.float8_e4m3

B, T, V, E, H = 4096, 32, 10000, 300, 300
NCORES = 8
BC = B // NCORES          # 512 batch rows per core
M = BC // 128             # 4 m-tiles
GC = 1200                 # 4*300 gate columns
CHUNKS = [(0, 512), (512, 1024), (1024, 1200)]
E_SPLITS = [(0, 128), (128, 256), (256, 301)]   # k-tiles of the [emb|1] contraction
H_SPLITS = [(0, 128), (128, 256), (256, 300)]   # h-dim splits for transposes/classifier
VTILES = (V + 127) // 128                        # 79

S_H = 16.0                # fp8 h scale
S_W = 32.0                # fp8 W_hh scale
S = S_H * S_W             # psum gate scale


def _raw(inst):
    return getattr(inst, "ins", inst)


def build_bass(steps, upad):
    nc = bacc.Bacc("TRN2", target_bir_lowering=False, debug=False, num_devices=NCORES)

    embT_d = nc.dram_tensor("embT", [E + 1, upad], bf16, kind="ExternalInput")
    wih_d = nc.dram_tensor("wih", [E + 1, GC], bf16, kind="ExternalInput")
    waug2_d = nc.dram_tensor("waug2", [128, 4, GC], fp8, kind="ExternalInput")
    wc_d = nc.dram_tensor("wc", [H, 2], bf16, kind="ExternalInput")
    clsb_d = nc.dram_tensor("clsb", [128, 2], f32, kind="ExternalInput")
    identb_d = nc.dram_tensor("identb", [128, 128], bf16, kind="ExternalInput")
    ident8_d = nc.dram_tensor("ident8", [128, 2, 128], fp8, kind="ExternalInput")
    idx_d = nc.dram_tensor("idx", [128, T, M], i32, kind="ExternalInput")
    mask_d = nc.dram_tensor("mask", [128, T, M], u8, kind="ExternalInput")
    G_d = nc.dram_tensor("G", [upad, GC], bf16, kind="Internal")
    out_d = nc.dram_tensor("out", [BC, 2], f32, kind="ExternalOutput")

    with tile.TileContext(nc, num_cores=NCORES) as tc:
        with (
            tc.tile_pool(name="const", bufs=1) as cpool,
            tc.tile_pool(name="state", bufs=1) as spool,
            tc.tile_pool(name="psum", bufs=2, space="PSUM") as ppool,
        ):
            # ---------- constants ----------
            identb = cpool.tile([128, 128], bf16, tag="identb")
            nc.sync.dma_start(out=identb[:, :], in_=identb_d[:, :])
            ident8 = cpool.tile([128, 2, 128], fp8, tag="ident8")
            nc.sync.dma_start(out=ident8[:, :, :], in_=ident8_d[:, :, :])
            waug2 = cpool.tile([128, 4, GC], fp8, tag="waug2")
            nc.sync.dma_start(out=waug2[:, :, :], in_=waug2_d[:, :, :])
            wc_sb = []
            for k, (d0, d1) in enumerate(H_SPLITS):
                t_ = cpool.tile([d1 - d0, 2], bf16, tag=f"wc{k}")
                nc.sync.dma_start(out=t_[:, :], in_=wc_d[d0:d1, :])
                wc_sb.append(t_)
            clsb = cpool.tile([128, 2], f32, tag="clsb")
            nc.sync.dma_start(out=clsb[:, :], in_=clsb_d[:, :])
            idx_sb = cpool.tile([128, T, M], i32, tag="idx")
            nc.sync.dma_start(out=idx_sb[:, :, :], in_=idx_d[:, :, :])
            mask_sb = cpool.tile([128, T, M], u8, tag="mask")
            nc.sync.dma_start(out=mask_sb[:, :, :], in_=mask_d[:, :, :])

            # ---------- state ----------
            # hT8 slots: [0]=h rows 0:128, [1]=128:256, [2]=256:300 (+garbage,
            # nulled by zero rows in waug2), [3]=zeros in waug2 -> don't care.
            hT8 = spool.tile([128, 4, BC], fp8, tag="hT8")
            nc.gpsimd.memset(hT8[:, :, :], 0.0)
            c_sb = spool.tile([128, M, H], bf16, tag="c")
            nc.gpsimd.memset(c_sb[:, :, :], 0.0)
            h_last = spool.tile([128, M, H], bf16, tag="hlast")
            hlT = []
            for k, (d0, d1) in enumerate(H_SPLITS):
                t_ = spool.tile([d1 - d0, BC], bf16, tag=f"hlT{k}")
                hlT.append(t_)

            # ---------- phase 1: G = [embT;1]^T @ [wih;b], scaled by S ----------
            g_stores = []
            with tc.tile_pool(name="gphase", bufs=1) as gpool, \
                 tc.tile_pool(name="gsbp", bufs=3) as gsbp:
                wih_sb = []
                for k, (d0, d1) in enumerate(E_SPLITS):
                    t_ = gpool.tile([d1 - d0, GC], bf16, tag=f"wih{k}")
                    nc.sync.dma_start(out=t_[:, :], in_=wih_d[d0:d1, :])
                    wih_sb.append(t_)
                embT_sb = []
                for k, (d0, d1) in enumerate(E_SPLITS):
                    t_ = gpool.tile([d1 - d0, upad], bf16, tag=f"emb{k}")
                    for q in range(4):
                        nc.sync.dma_start(
                            out=t_[:, q * (upad // 4):(q + 1) * (upad // 4)],
                            in_=embT_d[d0:d1, q * (upad // 4):(q + 1) * (upad // 4)])
                    embT_sb.append(t_)

                for v in range(upad // 128):
                    rows = 128
                    gps = ppool.tile([128, GC], f32, tag="gates")
                    for k, (d0, d1) in enumerate(E_SPLITS):
                        for (c0, c1) in CHUNKS:
                            nc.tensor.matmul(
                                gps[0:rows, c0:c1],
                                lhsT=embT_sb[k][:, v * 128:v * 128 + rows],
                                rhs=wih_sb[k][:, c0:c1],
                                start=(k == 0),
                                stop=(k == 2),
                            )
                    gsb = gsbp.tile([128, GC], bf16, tag="gsb")
                    nc.scalar.activation(
                        gsb[0:rows, 0:600], gps[0:rows, 0:600],
                        mybir.ActivationFunctionType.Copy, scale=S)
                    nc.vector.tensor_scalar(
                        out=gsb[0:rows, 600:1200], in0=gps[0:rows, 600:1200],
                        scalar1=S, scalar2=None, op0=mybir.AluOpType.mult)
                    st = nc.sync.dma_start(
                        out=G_d[v * 128:v * 128 + rows, :], in_=gsb[0:rows, :]
                    )
                    g_stores.append(st)

            g_done = nc.gpsimd.nop()
            for st in g_stores:
                tile.add_dep_helper(_raw(g_done), _raw(st), reason="G stored")

            # ---------- phase 2: recurrence ----------
            with tc.tile_pool(name="work", bufs=2) as wpool:
                for t in range(T):
                    ms = [m for m in range(M) if steps[m] > t]
                    xg = wpool.tile([128, M, GC], bf16, tag="xg")
                    for m in ms:
                        gather = nc.gpsimd.indirect_dma_start(
                            out=xg[:, m, :],
                            out_offset=None,
                            in_=G_d[:, :],
                            in_offset=bass.IndirectOffsetOnAxis(
                                ap=idx_sb[:, t, m:m + 1], axis=0),
                        )
                        tile.add_dep_helper(_raw(gather), _raw(g_done),
                                            reason="gather after G")

                    ifo = wpool.tile([128, M, 900], bf16, tag="ifo")
                    gt = wpool.tile([128, M, 300], bf16, tag="gt")
                    tch = wpool.tile([128, M, H], bf16, tag="tch")
                    hbf = wpool.tile([128, M, H], bf16, tag="hbf")
                    t1 = wpool.tile([128, M, H], bf16, tag="t1")
                    for m in ms:
                        gps = ppool.tile([128, GC], f32, tag="gates")
                        # inject xg (clears + seeds accumulation per chunk)
                        for (c0, c1) in CHUNKS:
                            nc.tensor.matmul(
                                gps[:, c0:c1],
                                lhsT=identb[:, :],
                                rhs=xg[:, m, c0:c1],
                                start=True,
                                stop=(t == 0),
                            )
                        if t > 0:
                            # gates += hT8^T @ waug2, fp8 DoubleRow (2 k-slot pairs)
                            for j, (s0, s1) in enumerate(((0, 2), (2, 4))):
                                for (c0, c1) in CHUNKS:
                                    nc.tensor.matmul(
                                        gps[:, c0:c1],
                                        lhsT=hT8[:, s0:s1, m * 128:(m + 1) * 128],
                                        rhs=waug2[:, s0:s1, c0:c1],
                                        start=False,
                                        stop=(j == 1),
                                        perf_mode=mybir.MatmulPerfMode.DoubleRow,
                                    )
                        nc.scalar.activation(
                            ifo[:, m, :], gps[:, 0:900],
                            mybir.ActivationFunctionType.Sigmoid, scale=1.0 / S)
                        nc.scalar.activation(
                            gt[:, m, :], gps[:, 900:1200],
                            mybir.ActivationFunctionType.Tanh, scale=1.0 / S)
                        # c = f*c + i*g ; h = o*tanh(c)
                        nc.vector.tensor_tensor(
                            out=t1[:, m, :], in0=ifo[:, m, 300:600],
                            in1=c_sb[:, m, :], op=mybir.AluOpType.mult)
                        nc.vector.tensor_tensor(
                            out=c_sb[:, m, :], in0=ifo[:, m, 0:300],
                            in1=gt[:, m, :], op=mybir.AluOpType.mult)
                        nc.vector.tensor_tensor(
                            out=c_sb[:, m, :], in0=c_sb[:, m, :],
                            in1=t1[:, m, :], op=mybir.AluOpType.add)
                        nc.scalar.activation(
                            tch[:, m, :], c_sb[:, m, :],
                            mybir.ActivationFunctionType.Tanh)
                        nc.vector.tensor_tensor(
                            out=hbf[:, m, :], in0=ifo[:, m, 600:900],
                            in1=tch[:, m, :], op=mybir.AluOpType.mult)
                        nc.vector.copy_predicated(
                            out=h_last[:, m, :],
                            mask=mask_sb[:, t, m:m + 1].to_broadcast([128, H]),
                            data=hbf[:, m, :])
                    for m in ms:
                        if t + 1 < steps[m]:
                            trp = ppool.tile([128, 3, 128], bf16, tag="tr")
                            for k, (d0, d1) in enumerate(H_SPLITS):
                                dk = d1 - d0
                                nc.tensor.transpose(
                                    out=trp[0:dk, k, :],
                                    in_=hbf[:, m, d0:d1],
                                    identity=identb[:, :])
                            # scaled fp8 convert during the psum->sbuf drain
                            nc.vector.tensor_scalar(
                                out=hT8[:, 0:2, m * 128:(m + 1) * 128],
                                in0=trp[:, 0:2, :], scalar1=S_H, scalar2=None,
                                op0=mybir.AluOpType.mult)
                            nc.scalar.activation(
                                hT8[0:44, 2, m * 128:(m + 1) * 128],
                                trp[0:44, 2, :],
                                mybir.ActivationFunctionType.Copy, scale=S_H)

                # ---------- phase 3: logits ----------
                trps = []
                for k in range(3):
                    trp = ppool.tile([128, 512], bf16, tag="gates")
                    trps.append(trp)
                for m in range(M):
                    for k, (d0, d1) in enumerate(H_SPLITS):
                        dk = d1 - d0
                        nc.tensor.transpose(
                            out=trps[k][0:dk, m * 128:(m + 1) * 128],
                            in_=h_last[:, m, d0:d1],
                            identity=identb[:, :])
                for k, (d0, d1) in enumerate(H_SPLITS):
                    dk = d1 - d0
                    nc.vector.tensor_copy(hlT[k][0:dk, :], trps[k][0:dk, :])

                lsb = wpool.tile([128, M, 2], f32, tag="lsb")
                for m in range(M):
                    lp = ppool.tile([128, 2], f32, tag="tr")
                    for k, (d0, d1) in enumerate(H_SPLITS):
                        nc.tensor.matmul(
                            lp[:, :],
                            lhsT=hlT[k][:, m * 128:(m + 1) * 128],
                            rhs=wc_sb[k][:, :],
                            start=(k == 0),
                            stop=(k == 2),
                        )
                    nc.vector.scalar_tensor_tensor(
                        out=lsb[:, m, :], in0=lp[:, :], scalar=1.0,
                        in1=clsb[:, :],
                        op0=mybir.AluOpType.mult, op1=mybir.AluOpType.add)
                nc.sync.dma_start(
                    out=out_d[:, :].rearrange("(m p) c -> p m c", p=128),
                    in_=lsb[:, :, :])

    nc.compile()
    return nc


_NC_CACHE = {}
LAST_RESULT = None


def _host_prep(inputs):
    cap = np.asarray(inputs["cap"]).astype(np.int64)
    cap_len = np.asarray(inputs["cap_len"]).astype(np.int64)
    embed_w = np.asarray(inputs["embed_w"], dtype=np.float32)
    W_ih = np.asarray(inputs["W_ih"], dtype=np.float32)
    W_hh = np.asarray(inputs["W_hh"], dtype=np.float32)
    b = (np.asarray(inputs["b_ih"], dtype=np.float32)
         + np.asarray(inputs["b_hh"], dtype=np.float32))
    cls_v = np.asarray(inputs["cls_v"], dtype=np.float32)
    cls_g = np.asarray(inputs["cls_g"], dtype=np.float32)
    cls_b = np.asarray(inputs["cls_b"], dtype=np.float32)

    # gate order [i f o g]
    perm = np.concatenate([np.arange(0, 300), np.arange(300, 600),
                           np.arange(900, 1200), np.arange(600, 900)])
    wih_aug = np.zeros((E + 1, GC), np.float32)
    wih_aug[:E] = W_ih[perm].T
    wih_aug[E] = b[perm]
    embT_aug = np.ones((E + 1, V), np.float32)
    embT_aug[:E] = embed_w.T

    Wp = W_hh[perm].T * S_W                          # [300, 1200], scaled
    waug2 = np.zeros((128, 4, GC), np.float32)
    waug2[:, 0, :] = Wp[0:128]
    waug2[:, 1, :] = Wp[128:256]
    waug2[0:44, 2, :] = Wp[256:300]

    Wc = cls_g * cls_v / np.linalg.norm(cls_v, axis=1, keepdims=True)  # [2, 300]

    # global sort by length; deal round-robin to cores
    order = np.argsort(cap_len, kind="stable")
    steps = []
    for m in range(M):
        mx = 0
        for c in range(NCORES):
            sel = order[c::NCORES]
            mx = max(mx, int(cap_len[sel[m * 128:(m + 1) * 128]].max()))
        steps.append(mx)
    steps = tuple(steps)

    # per-core used-vocab compaction
    used_l, inv_l = [], []
    for c in range(NCORES):
        sel = order[c::NCORES]
        used = np.unique(cap[sel])
        inv = np.zeros(V, np.int64)
        inv[used] = np.arange(len(used))
        used_l.append(used)
        inv_l.append(inv)
    upad = -(-max(len(u) for u in used_l) // 512) * 512

    shared = {
        "identb": np.eye(128, dtype=np.float32).astype(np_bf16),
        "ident8": np.broadcast_to(
            np.eye(128, dtype=np.float32)[:, None, :], (128, 2, 128)
        ).astype(np_fp8),
        "wih": wih_aug.astype(np_bf16),
        "waug2": waug2.astype(np_fp8),
        "wc": Wc.T.astype(np_bf16),
        "clsb": np.tile(cls_b.reshape(1, 2), (128, 1)).astype(np.float32),
    }
    in_maps = []
    for core in range(NCORES):
        sel = order[core::NCORES]
        capc = inv_l[core][cap[sel]]                   # [512, 32] remapped
        lenc = cap_len[sel]                            # [512]
        cols = np.zeros(upad, np.int64)
        cols[:len(used_l[core])] = used_l[core]
        embT_c = embT_aug[:, cols].astype(np_bf16)
        idx = np.ascontiguousarray(
            capc.reshape(M, 128, T).transpose(1, 2, 0)).astype(np.int32)
        lm = lenc.reshape(M, 128).T                    # [128, M]
        mask = (lm[:, None, :] - 1 == np.arange(T)[None, :, None]).astype(np.uint8)
        in_maps.append(dict(shared, idx=idx, embT=embT_c,
                            mask=np.ascontiguousarray(mask)))
    return in_maps, order, steps, upad


def kernel(**inputs) -> np.ndarray:
    global LAST_RESULT
    from concourse.bass_utils import run_bass_kernel_spmd
    in_maps, order, steps, upad = _host_prep(inputs)
    if (steps, upad) not in _NC_CACHE:
        _NC_CACHE[(steps, upad)] = build_bass(steps, upad)
    nc = _NC_CACHE[(steps, upad)]
    trace = bool(int(os.environ.get("KERNEL_TRACE", "0")))
    res = run_bass_kernel_spmd(nc, in_maps, core_ids=list(range(NCORES)), trace=trace)
    LAST_RESULT = res
    out = np.empty((B, 2), np.float32)
    for core in range(NCORES):
        out[order[core::NCORES]] = res.results[core]["out"].astype(np.float32)
    return out


# revision 19
# speedup vs baseline: 1.5593x; 1.0108x over previous
"""Trainium2 Bass kernel for nn_Discriminator_lstm (B=4096, T=32, E=H=300, VOCAB=10000).

Strategy (data-parallel over batch, 8 cores x 512 rows):
  Host: globally sort rows by cap_len, deal ranks round-robin to cores
        (every core gets the same length distribution), sorted ascending
        within each core.  m-tile m then has max length steps[m] (~8/16/24/32),
        and the recurrence runs only steps[m] steps for tile m.
  Phase 1 (per core): G = [embT;1] ^T @ [wih;b] in bf16 -> DRAM scratch
        [10000, 1200], scaled by S=512 (bias folded in via ones-row).
  Phase 2: per live (t, m):
        - indirect-DMA gather xg = G[cap[:, t]] (bf16) into SBUF
        - PE: inject xg into PSUM (identity matmul, bf16), then accumulate
          gates += hT8^T @ waug2 as TWO fp8e4m3 DoubleRow matmuls
          (k-slots [0:2] and [2:4]; h scaled by s_h=16, W_hh by s_w=32,
          psum scale S = s_h*s_w = 512)
        - ACT: sigmoid/tanh with scale=1/S -> bf16
        - DVE: c = f*c + i*g (f32); h_bf = o*tanh(c); masked h_last capture
        - Pool: h8 = (o*s_h)*tanh(c) -> fp8 for the next step's matmul
        - PE: transpose h8 (fp8, 1 cyc/row) -> one merged DVE copy into hT8
  Phase 3: logits = h_last @ Wc^T + cls_b in bf16.
"""

import os
import sys

import numpy as np

for _p in ("/opt/trn_rl_repo", "/root/.axon_site/_ro/trn_rl_repo"):
    if os.path.isdir(_p) and _p not in sys.path:
        sys.path.insert(0, _p)

import ml_dtypes

import concourse.bass as bass
import concourse.bacc as bacc
import concourse.mybir as mybir
import concourse.tile as tile

f32 = mybir.dt.float32
bf16 = mybir.dt.bfloat16
fp8 = mybir.dt.float8e4
i32 = mybir.dt.int32
u8 = mybir.dt.uint8

np_bf16 = ml_dtypes.bfloat16
np_fp8 = ml_dtypes.float8_e4m3

B, T, V, E, H = 4096, 32, 10000, 300, 300
NCORES = 8
BC = B // NCORES          # 512 batch rows per core
M = BC // 128             # 4 m-tiles
GC = 1200                 # 4*300 gate columns
CHUNKS = [(0, 512), (512, 1024), (1024, 1200)]
E_SPLITS = [(0, 128), (128, 256), (256, 301)]   # k-tiles of the [emb|1] contraction
H_SPLITS = [(0, 128), (128, 256), (256, 300)]   # h-dim splits for transposes/classifier
VTILES = (V + 127) // 128                        # 79

S_H = 16.0                # fp8 h scale
S_W = 32.0                # fp8 W_hh scale
S = S_H * S_W             # psum gate scale


def _raw(inst):
    return getattr(inst, "ins", inst)


def build_bass(steps, upad):
    nc = bacc.Bacc("TRN2", target_bir_lowering=False, debug=False, num_devices=NCORES)

    embT_d = nc.dram_tensor("embT", [E + 1, upad], bf16, kind="ExternalInput")
    wih_d = nc.dram_tensor("wih", [E + 1, GC], bf16, kind="ExternalInput")
    waug2_d = nc.dram_tensor("waug2", [128, 4, GC], fp8, kind="ExternalInput")
    wc_d = nc.dram_tensor("wc", [H, 2], bf16, kind="ExternalInput")
    clsb_d = nc.dram_tensor("clsb", [128, 2], f32, kind="ExternalInput")
    identb_d = nc.dram_tensor("identb", [128, 128], bf16, kind="ExternalInput")
    ident8_d = nc.dram_tensor("ident8", [128, 2, 128], fp8, kind="ExternalInput")
    idx_d = nc.dram_tensor("idx", [128, T, M], i32, kind="ExternalInput")
    mask_d = nc.dram_tensor("mask", [128, T, M], u8, kind="ExternalInput")
    G_d = nc.dram_tensor("G", [upad, GC], bf16, kind="Internal")
    out_d = nc.dram_tensor("out", [BC, 2], f32, kind="ExternalOutput")

    with tile.TileContext(nc, num_cores=NCORES) as tc:
        with (
            tc.tile_pool(name="const", bufs=1) as cpool,
            tc.tile_pool(name="state", bufs=1) as spool,
            tc.tile_pool(name="psum", bufs=2, space="PSUM") as ppool,
        ):
            # ---------- constants ----------
            identb = cpool.tile([128, 128], bf16, tag="identb")
            nc.sync.dma_start(out=identb[:, :], in_=identb_d[:, :])
            ident8 = cpool.tile([128, 2, 128], fp8, tag="ident8")
            nc.sync.dma_start(out=ident8[:, :, :], in_=ident8_d[:, :, :])
            waug2 = cpool.tile([128, 4, GC], fp8, tag="waug2")
            nc.sync.dma_start(out=waug2[:, :, :], in_=waug2_d[:, :, :])
            wc_sb = []
            for k, (d0, d1) in enumerate(H_SPLITS):
                t_ = cpool.tile([d1 - d0, 2], bf16, tag=f"wc{k}")
                nc.sync.dma_start(out=t_[:, :], in_=wc_d[d0:d1, :])
                wc_sb.append(t_)
            clsb = cpool.tile([128, 2], f32, tag="clsb")
            nc.sync.dma_start(out=clsb[:, :], in_=clsb_d[:, :])
            idx_sb = cpool.tile([128, T, M], i32, tag="idx")
            nc.sync.dma_start(out=idx_sb[:, :, :], in_=idx_d[:, :, :])
            mask_sb = cpool.tile([128, T, M], u8, tag="mask")
            nc.sync.dma_start(out=mask_sb[:, :, :], in_=mask_d[:, :, :])

            # ---------- state ----------
            # hT8 slots: [0]=h rows 0:128, [1]=128:256, [2]=256:300 (+garbage,
            # nulled by zero rows in waug2), [3]=zeros in waug2 -> don't care.
            hT8 = spool.tile([128, 4, BC], fp8, tag="hT8")
            nc.gpsimd.memset(hT8[:, :, :], 0.0)
            c_sb = spool.tile([128, M, H], bf16, tag="c")
            nc.gpsimd.memset(c_sb[:, :, :], 0.0)
            h_last = spool.tile([128, M, H], bf16, tag="hlast")
            hlT = []
            for k, (d0, d1) in enumerate(H_SPLITS):
                t_ = spool.tile([d1 - d0, BC], bf16, tag=f"hlT{k}")
                hlT.append(t_)

            # ---------- phase 1: G = [embT;1]^T @ [wih;b], scaled by S ----------
            g_stores = []
            with tc.tile_pool(name="gphase", bufs=1) as gpool, \
                 tc.tile_pool(name="gsbp", bufs=3) as gsbp:
                wih_sb = []
                for k, (d0, d1) in enumerate(E_SPLITS):
                    t_ = gpool.tile([d1 - d0, GC], bf16, tag=f"wih{k}")
                    nc.sync.dma_start(out=t_[:, :], in_=wih_d[d0:d1, :])
                    wih_sb.append(t_)
                embT_sb = []
                for k, (d0, d1) in enumerate(E_SPLITS):
                    t_ = gpool.tile([d1 - d0, upad], bf16, tag=f"emb{k}")
                    for q in range(4):
                        nc.sync.dma_start(
                            out=t_[:, q * (upad // 4):(q + 1) * (upad // 4)],
                            in_=embT_d[d0:d1, q * (upad // 4):(q + 1) * (upad // 4)])
                    embT_sb.append(t_)

                for v in range(upad // 128):
                    rows = 128
                    gps = ppool.tile([128, GC], f32, tag="gates")
                    for k, (d0, d1) in enumerate(E_SPLITS):
                        for (c0, c1) in CHUNKS:
                            nc.tensor.matmul(
                                gps[0:rows, c0:c1],
                                lhsT=embT_sb[k][:, v * 128:v * 128 + rows],
                                rhs=wih_sb[k][:, c0:c1],
                                start=(k == 0),
                                stop=(k == 2),
                            )
                    gsb = gsbp.tile([128, GC], bf16, tag="gsb")
                    nc.scalar.activation(
                        gsb[0:rows, 0:600], gps[0:rows, 0:600],
                        mybir.ActivationFunctionType.Copy, scale=S)
                    nc.vector.tensor_scalar(
                        out=gsb[0:rows, 600:1200], in0=gps[0:rows, 600:1200],
                        scalar1=S, scalar2=None, op0=mybir.AluOpType.mult)
                    st = nc.sync.dma_start(
                        out=G_d[v * 128:v * 128 + rows, :], in_=gsb[0:rows, :]
                    )
                    g_stores.append(st)

            g_done = nc.gpsimd.nop()
            for st in g_stores:
                tile.add_dep_helper(_raw(g_done), _raw(st), reason="G stored")

            # ---------- phase 2: recurrence ----------
            with tc.tile_pool(name="work", bufs=2) as wpool:
                for t in range(T):
                    ms = [m for m in range(M) if steps[m] > t]
                    xg = wpool.tile([128, M, GC], bf16, tag="xg")
                    for m in ms:
                        gather = nc.gpsimd.indirect_dma_start(
                            out=xg[:, m, :],
                            out_offset=None,
                            in_=G_d[:, :],
                            in_offset=bass.IndirectOffsetOnAxis(
                                ap=idx_sb[:, t, m:m + 1], axis=0),
                        )
                        tile.add_dep_helper(_raw(gather), _raw(g_done),
                                            reason="gather after G")

                    ifo = wpool.tile([128, M, 900], bf16, tag="ifo")
                    gt = wpool.tile([128, M, 300], bf16, tag="gt")
                    tch = wpool.tile([128, M, H], bf16, tag="tch")
                    hbf = wpool.tile([128, M, H], bf16, tag="hbf")
                    t1 = wpool.tile([128, M, H], bf16, tag="t1")
                    for m in ms:
                        gps = ppool.tile([128, GC], f32, tag="gates")
                        # inject xg (clears + seeds accumulation per chunk)
                        for (c0, c1) in CHUNKS:
                            nc.tensor.matmul(
                                gps[:, c0:c1],
                                lhsT=identb[:, :],
                                rhs=xg[:, m, c0:c1],
                                start=True,
                                stop=(t == 0),
                            )
                        if t > 0:
                            # gates += hT8^T @ waug2, fp8 DoubleRow (2 k-slot pairs)
                            for j, (s0, s1) in enumerate(((0, 2), (2, 4))):
                                for (c0, c1) in CHUNKS:
                                    nc.tensor.matmul(
                                        gps[:, c0:c1],
                                        lhsT=hT8[:, s0:s1, m * 128:(m + 1) * 128],
                                        rhs=waug2[:, s0:s1, c0:c1],
                                        start=False,
                                        stop=(j == 1),
                                        perf_mode=mybir.MatmulPerfMode.DoubleRow,
                                    )
                        nc.scalar.activation(
                            ifo[:, m, :], gps[:, 0:900],
                            mybir.ActivationFunctionType.Sigmoid, scale=1.0 / S)
                        nc.scalar.activation(
                            gt[:, m, :], gps[:, 900:1200],
                            mybir.ActivationFunctionType.Tanh, scale=1.0 / S)
                        # c = f*c + i*g ; h = o*tanh(c)
                        nc.vector.tensor_tensor(
                            out=t1[:, m, :], in0=ifo[:, m, 300:600],
                            in1=c_sb[:, m, :], op=mybir.AluOpType.mult)
                        nc.vector.tensor_tensor(
                            out=c_sb[:, m, :], in0=ifo[:, m, 0:300],
                            in1=gt[:, m, :], op=mybir.AluOpType.mult)
                        nc.vector.tensor_tensor(
                            out=c_sb[:, m, :], in0=c_sb[:, m, :],
                            in1=t1[:, m, :], op=mybir.AluOpType.add)
                        nc.scalar.activation(
                            tch[:, m, :], c_sb[:, m, :],
                            mybir.ActivationFunctionType.Tanh)
                        nc.vector.tensor_tensor(
                            out=hbf[:, m, :], in0=ifo[:, m, 600:900],
                            in1=tch[:, m, :], op=mybir.AluOpType.mult)
                        nc.vector.copy_predicated(
                            out=h_last[:, m, :],
                            mask=mask_sb[:, t, m:m + 1].to_broadcast([128, H]),
                            data=hbf[:, m, :])
                    for m in ms:
                        if t + 1 < steps[m]:
                            trp = ppool.tile([128, 3, 128], bf16, tag="tr")
                            for k, (d0, d1) in enumerate(H_SPLITS):
                                dk = d1 - d0
                                nc.tensor.transpose(
                                    out=trp[0:dk, k, :],
                                    in_=hbf[:, m, d0:d1],
                                    identity=identb[:, :])
                            # scaled fp8 convert during the psum->sbuf drain
                            nc.vector.tensor_scalar(
                                out=hT8[:, 0:2, m * 128:(m + 1) * 128],
                                in0=trp[:, 0:2, :], scalar1=S_H, scalar2=None,
                                op0=mybir.AluOpType.mult)
                            nc.vector.tensor_scalar(
                                out=hT8[0:44, 2, m * 128:(m + 1) * 128],
                                in0=trp[0:44, 2, :], scalar1=S_H, scalar2=None,
                                op0=mybir.AluOpType.mult)

                # ---------- phase 3: logits ----------
                trps = []
                for k in range(3):
                    trp = ppool.tile([128, 512], bf16, tag="gates")
                    trps.append(trp)
                for m in range(M):
                    for k, (d0, d1) in enumerate(H_SPLITS):
                        dk = d1 - d0
                        nc.tensor.transpose(
                            out=trps[k][0:dk, m * 128:(m + 1) * 128],
                            in_=h_last[:, m, d0:d1],
                            identity=identb[:, :])
                for k, (d0, d1) in enumerate(H_SPLITS):
                    dk = d1 - d0
                    nc.vector.tensor_copy(hlT[k][0:dk, :], trps[k][0:dk, :])

                lsb = wpool.tile([128, M, 2], f32, tag="lsb")
                for m in range(M):
                    lp = ppool.tile([128, 2], f32, tag="tr")
                    for k, (d0, d1) in enumerate(H_SPLITS):
                        nc.tensor.matmul(
                            lp[:, :],
                            lhsT=hlT[k][:, m * 128:(m + 1) * 128],
                            rhs=wc_sb[k][:, :],
                            start=(k == 0),
                            stop=(k == 2),
                        )
                    nc.vector.scalar_tensor_tensor(
                        out=lsb[:, m, :], in0=lp[:, :], scalar=1.0,
                        in1=clsb[:, :],
                        op0=mybir.AluOpType.mult, op1=mybir.AluOpType.add)
                nc.sync.dma_start(
                    out=out_d[:, :].rearrange("(m p) c -> p m c", p=128),
                    in_=lsb[:, :, :])

    nc.compile()
    return nc


_NC_CACHE = {}
LAST_RESULT = None


def _host_prep(inputs):
    cap = np.asarray(inputs["cap"]).astype(np.int64)
    cap_len = np.asarray(inputs["cap_len"]).astype(np.int64)
    embed_w = np.asarray(inputs["embed_w"], dtype=np.float32)
    W_ih = np.asarray(inputs["W_ih"], dtype=np.float32)
    W_hh = np.asarray(inputs["W_hh"], dtype=np.float32)
    b = (np.asarray(inputs["b_ih"], dtype=np.float32)
         + np.asarray(inputs["b_hh"], dtype=np.float32))
    cls_v = np.asarray(inputs["cls_v"], dtype=np.float32)
    cls_g = np.asarray(inputs["cls_g"], dtype=np.float32)
    cls_b = np.asarray(inputs["cls_b"], dtype=np.float32)

    # gate order [i f o g]
    perm = np.concatenate([np.arange(0, 300), np.arange(300, 600),
                           np.arange(900, 1200), np.arange(600, 900)])
    wih_aug = np.zeros((E + 1, GC), np.float32)
    wih_aug[:E] = W_ih[perm].T
    wih_aug[E] = b[perm]
    embT_aug = np.ones((E + 1, V), np.float32)
    embT_aug[:E] = embed_w.T

    Wp = W_hh[perm].T * S_W                          # [300, 1200], scaled
    waug2 = np.zeros((128, 4, GC), np.float32)
    waug2[:, 0, :] = Wp[0:128]
    waug2[:, 1, :] = Wp[128:256]
    waug2[0:44, 2, :] = Wp[256:300]

    Wc = cls_g * cls_v / np.linalg.norm(cls_v, axis=1, keepdims=True)  # [2, 300]

    # global sort by length; deal round-robin to cores
    order = np.argsort(cap_len, kind="stable")
    steps = []
    for m in range(M):
        mx = 0
        for c in range(NCORES):
            sel = order[c::NCORES]
            mx = max(mx, int(cap_len[sel[m * 128:(m + 1) * 128]].max()))
        steps.append(mx)
    steps = tuple(steps)

    # per-core used-vocab compaction
    used_l, inv_l = [], []
    for c in range(NCORES):
        sel = order[c::NCORES]
        used = np.unique(cap[sel])
        inv = np.zeros(V, np.int64)
        inv[used] = np.arange(len(used))
        used_l.append(used)
        inv_l.append(inv)
    upad = -(-max(len(u) for u in used_l) // 512) * 512

    shared = {
        "identb": np.eye(128, dtype=np.float32).astype(np_bf16),
        "ident8": np.broadcast_to(
            np.eye(128, dtype=np.float32)[:, None, :], (128, 2, 128)
        ).astype(np_fp8),
        "wih": wih_aug.astype(np_bf16),
        "waug2": waug2.astype(np_fp8),
        "wc": Wc.T.astype(np_bf16),
        "clsb": np.tile(cls_b.reshape(1, 2), (128, 1)).astype(np.float32),
    }
    in_maps = []
    for core in range(NCORES):
        sel = order[core::NCORES]
        capc = inv_l[core][cap[sel]]                   # [512, 32] remapped
        lenc = cap_len[sel]                            # [512]
        cols = np.zeros(upad, np.int64)
        cols[:len(used_l[core])] = used_l[core]
        embT_c = embT_aug[:, cols].astype(np_bf16)
        idx = np.ascontiguousarray(
            capc.reshape(M, 128, T).transpose(1, 2, 0)).astype(np.int32)
        lm = lenc.reshape(M, 128).T                    # [128, M]
        mask = (lm[:, None, :] - 1 == np.arange(T)[None, :, None]).astype(np.uint8)
        in_maps.append(dict(shared, idx=idx, embT=embT_c,
                            mask=np.ascontiguousarray(mask)))
    return in_maps, order, steps, upad


def kernel(**inputs) -> np.ndarray:
    global LAST_RESULT
    from concourse.bass_utils import run_bass_kernel_spmd
    in_maps, order, steps, upad = _host_prep(inputs)
    if (steps, upad) not in _NC_CACHE:
        _NC_CACHE[(steps, upad)] = build_bass(steps, upad)
    nc = _NC_CACHE[(steps, upad)]
    trace = bool(int(os.environ.get("KERNEL_TRACE", "0")))
    res = run_bass_kernel_spmd(nc, in_maps, core_ids=list(range(NCORES)), trace=trace)
    LAST_RESULT = res
    out = np.empty((B, 2), np.float32)
    for core in range(NCORES):
        out[order[core::NCORES]] = res.results[core]["out"].astype(np.float32)
    return out


# revision 27
# speedup vs baseline: 1.6160x; 1.0364x over previous
"""Trainium2 Bass kernel for nn_Discriminator_lstm (B=4096, T=32, E=H=300, VOCAB=10000).

Strategy (data-parallel over batch, 8 cores x 512 rows):
  Host: globally sort rows by cap_len, deal ranks round-robin to cores
        (every core gets the same length distribution), sorted ascending
        within each core.  m-tile m then has max length steps[m] (~8/16/24/32),
        and the recurrence runs only steps[m] steps for tile m.
  Phase 1 (per core): G = [embT;1] ^T @ [wih;b] in bf16 -> DRAM scratch
        [10000, 1200], scaled by S=512 (bias folded in via ones-row).
  Phase 2: per live (t, m):
        - indirect-DMA gather xg = G[cap[:, t]] (bf16) into SBUF
        - PE: inject xg into PSUM (identity matmul, bf16), then accumulate
          gates += hT8^T @ waug2 as TWO fp8e4m3 DoubleRow matmuls
          (k-slots [0:2] and [2:4]; h scaled by s_h=16, W_hh by s_w=32,
          psum scale S = s_h*s_w = 512)
        - ACT: sigmoid/tanh with scale=1/S -> bf16
        - DVE: c = f*c + i*g (f32); h_bf = o*tanh(c); masked h_last capture
        - Pool: h8 = (o*s_h)*tanh(c) -> fp8 for the next step's matmul
        - PE: transpose h8 (fp8, 1 cyc/row) -> one merged DVE copy into hT8
  Phase 3: logits = h_last @ Wc^T + cls_b in bf16.
"""

import os
import sys

import numpy as np

for _p in ("/opt/trn_rl_repo", "/root/.axon_site/_ro/trn_rl_repo"):
    if os.path.isdir(_p) and _p not in sys.path:
        sys.path.insert(0, _p)

import ml_dtypes

import concourse.bass as bass
import concourse.bacc as bacc
import concourse.mybir as mybir
import concourse.tile as tile

f32 = mybir.dt.float32
bf16 = mybir.dt.bfloat16
fp8 = mybir.dt.float8e4
i32 = mybir.dt.int32
u8 = mybir.dt.uint8

np_bf16 = ml_dtypes.bfloat16
np_fp8 = ml_dtypes.float8_e4m3

B, T, V, E, H = 4096, 32, 10000, 300, 300
NCORES = 8
BC = B // NCORES          # 512 batch rows per core
M = BC // 128             # 4 m-tiles
GC = 1200                 # 4*300 gate columns
CHUNKS = [(0, 512), (512, 1024), (1024, 1200)]
E_SPLITS = [(0, 128), (128, 256), (256, 301)]   # k-tiles of the [emb|1] contraction
H_SPLITS = [(0, 128), (128, 256), (256, 300)]   # h-dim splits for transposes/classifier
VTILES = (V + 127) // 128                        # 79

S_H = 16.0                # fp8 h scale
S_W = 32.0                # fp8 W_hh scale
S = S_H * S_W             # psum gate scale


def _raw(inst):
    return getattr(inst, "ins", inst)


def build_bass(steps, upad):
    nc = bacc.Bacc("TRN2", target_bir_lowering=False, debug=False, num_devices=NCORES)

    embT_d = nc.dram_tensor("embT", [E + 1, upad], bf16, kind="ExternalInput")
    wih_d = nc.dram_tensor("wih", [E + 1, GC], bf16, kind="ExternalInput")
    waug2_d = nc.dram_tensor("waug2", [128, 4, GC], fp8, kind="ExternalInput")
    wc_d = nc.dram_tensor("wc", [H, 2], bf16, kind="ExternalInput")
    clsb_d = nc.dram_tensor("clsb", [128, 2], f32, kind="ExternalInput")
    identb_d = nc.dram_tensor("identb", [128, 128], bf16, kind="ExternalInput")
    ident8_d = nc.dram_tensor("ident8", [128, 2, 128], fp8, kind="ExternalInput")
    idx_d = nc.dram_tensor("idx", [128, T, M], i32, kind="ExternalInput")
    mask_d = nc.dram_tensor("mask", [128, T, M], u8, kind="ExternalInput")
    G_d = nc.dram_tensor("G", [upad, GC], bf16, kind="Internal")
    out_d = nc.dram_tensor("out", [BC, 2], f32, kind="ExternalOutput")

    with tile.TileContext(nc, num_cores=NCORES) as tc:
        with (
            tc.tile_pool(name="const", bufs=1) as cpool,
            tc.tile_pool(name="state", bufs=1) as spool,
            tc.tile_pool(name="psum", bufs=2, space="PSUM") as ppool,
        ):
            # ---------- constants ----------
            identb = cpool.tile([128, 128], bf16, tag="identb")
            nc.sync.dma_start(out=identb[:, :], in_=identb_d[:, :])
            ident8 = cpool.tile([128, 2, 128], fp8, tag="ident8")
            nc.sync.dma_start(out=ident8[:, :, :], in_=ident8_d[:, :, :])
            waug2 = cpool.tile([128, 4, GC], fp8, tag="waug2")
            nc.sync.dma_start(out=waug2[:, :, :], in_=waug2_d[:, :, :])
            wc_sb = []
            for k, (d0, d1) in enumerate(H_SPLITS):
                t_ = cpool.tile([d1 - d0, 2], bf16, tag=f"wc{k}")
                nc.sync.dma_start(out=t_[:, :], in_=wc_d[d0:d1, :])
                wc_sb.append(t_)
            clsb = cpool.tile([128, 2], f32, tag="clsb")
            nc.sync.dma_start(out=clsb[:, :], in_=clsb_d[:, :])
            idx_sb = cpool.tile([128, T, M], i32, tag="idx")
            nc.sync.dma_start(out=idx_sb[:, :, :], in_=idx_d[:, :, :])
            mask_sb = cpool.tile([128, T, M], u8, tag="mask")
            nc.sync.dma_start(out=mask_sb[:, :, :], in_=mask_d[:, :, :])

            # ---------- state ----------
            # hT8 slots: [0]=h rows 0:128, [1]=128:256, [2]=256:300 (+garbage,
            # nulled by zero rows in waug2), [3]=zeros in waug2 -> don't care.
            hT8 = spool.tile([128, 4, BC], fp8, tag="hT8")
            nc.gpsimd.memset(hT8[:, :, :], 0.0)
            c_sb = spool.tile([128, M, H], bf16, tag="c")
            nc.gpsimd.memset(c_sb[:, :, :], 0.0)
            h_last = spool.tile([128, M, H], bf16, tag="hlast")
            hlT = []
            for k, (d0, d1) in enumerate(H_SPLITS):
                t_ = spool.tile([d1 - d0, BC], bf16, tag=f"hlT{k}")
                hlT.append(t_)

            # ---------- phase 1: G = [embT;1]^T @ [wih;b], scaled by S ----------
            g_stores = []
            with tc.tile_pool(name="gphase", bufs=1) as gpool, \
                 tc.tile_pool(name="gsbp", bufs=12) as gsbp:
                wih_sb = []
                for k, (d0, d1) in enumerate(E_SPLITS):
                    t_ = gpool.tile([d1 - d0, GC], bf16, tag=f"wih{k}")
                    nc.sync.dma_start(out=t_[:, :], in_=wih_d[d0:d1, :])
                    wih_sb.append(t_)
                embT_sb = []
                for k, (d0, d1) in enumerate(E_SPLITS):
                    t_ = gpool.tile([d1 - d0, upad], bf16, tag=f"emb{k}")
                    embT_sb.append(t_)
                # column-major load order so early vtiles unblock quickly
                NQ = 8
                for q in range(NQ):
                    q0, q1 = q * (upad // NQ), (q + 1) * (upad // NQ)
                    for k, (d0, d1) in enumerate(E_SPLITS):
                        nc.sync.dma_start(
                            out=embT_sb[k][:, q0:q1],
                            in_=embT_d[d0:d1, q0:q1])

                for v in range(upad // 128):
                    rows = 128
                    gps = ppool.tile([128, GC], f32, tag="gates")
                    for k, (d0, d1) in enumerate(E_SPLITS):
                        for (c0, c1) in CHUNKS:
                            nc.tensor.matmul(
                                gps[0:rows, c0:c1],
                                lhsT=embT_sb[k][:, v * 128:v * 128 + rows],
                                rhs=wih_sb[k][:, c0:c1],
                                start=(k == 0),
                                stop=(k == 2),
                            )
                    gsb = gsbp.tile([128, GC], bf16, tag="gsb")
                    nc.scalar.activation(
                        gsb[0:rows, 0:600], gps[0:rows, 0:600],
                        mybir.ActivationFunctionType.Copy, scale=S)
                    nc.vector.tensor_scalar(
                        out=gsb[0:rows, 600:1200], in0=gps[0:rows, 600:1200],
                        scalar1=S, scalar2=None, op0=mybir.AluOpType.mult)
                    st = nc.sync.dma_start(
                        out=G_d[v * 128:v * 128 + rows, :], in_=gsb[0:rows, :]
                    )
                    g_stores.append(st)

            g_done = nc.gpsimd.nop()
            for st in g_stores:
                tile.add_dep_helper(_raw(g_done), _raw(st), reason="G stored")

            # ---------- phase 2: recurrence ----------
            with tc.tile_pool(name="work", bufs=2) as wpool:
                for t in range(T):
                    ms = [m for m in range(M) if steps[m] > t]
                    xg = wpool.tile([128, M, GC], bf16, tag="xg")
                    for m in ms:
                        gather = nc.gpsimd.indirect_dma_start(
                            out=xg[:, m, :],
                            out_offset=None,
                            in_=G_d[:, :],
                            in_offset=bass.IndirectOffsetOnAxis(
                                ap=idx_sb[:, t, m:m + 1], axis=0),
                        )
                        tile.add_dep_helper(_raw(gather), _raw(g_done),
                                            reason="gather after G")

                    ifo = wpool.tile([128, M, 900], bf16, tag="ifo")
                    gt = wpool.tile([128, M, 300], bf16, tag="gt")
                    tch = wpool.tile([128, M, H], bf16, tag="tch")
                    hbf = wpool.tile([128, M, H], bf16, tag="hbf")
                    t1 = wpool.tile([128, M, H], bf16, tag="t1")
                    for m in ms:
                        gps = ppool.tile([128, GC], f32, tag="gates")
                        # inject xg (clears + seeds accumulation per chunk)
                        for (c0, c1) in CHUNKS:
                            nc.tensor.matmul(
                                gps[:, c0:c1],
                                lhsT=identb[:, :],
                                rhs=xg[:, m, c0:c1],
                                start=True,
                                stop=(t == 0),
                            )
                        if t > 0:
                            # gates += hT8^T @ waug2, fp8 DoubleRow (2 k-slot pairs)
                            for j, (s0, s1) in enumerate(((0, 2), (2, 4))):
                                for (c0, c1) in CHUNKS:
                                    nc.tensor.matmul(
                                        gps[:, c0:c1],
                                        lhsT=hT8[:, s0:s1, m * 128:(m + 1) * 128],
                                        rhs=waug2[:, s0:s1, c0:c1],
                                        start=False,
                                        stop=(j == 1),
                                        perf_mode=mybir.MatmulPerfMode.DoubleRow,
                                    )
                        nc.scalar.activation(
                            ifo[:, m, :], gps[:, 0:900],
                            mybir.ActivationFunctionType.Sigmoid, scale=1.0 / S)
                        nc.scalar.activation(
                            gt[:, m, :], gps[:, 900:1200],
                            mybir.ActivationFunctionType.Tanh, scale=1.0 / S)
                        # c = f*c + i*g ; h = o*tanh(c)
                        nc.vector.tensor_tensor(
                            out=t1[:, m, :], in0=ifo[:, m, 300:600],
                            in1=c_sb[:, m, :], op=mybir.AluOpType.mult)
                        nc.vector.tensor_tensor(
                            out=c_sb[:, m, :], in0=ifo[:, m, 0:300],
                            in1=gt[:, m, :], op=mybir.AluOpType.mult)
                        nc.vector.tensor_tensor(
                            out=c_sb[:, m, :], in0=c_sb[:, m, :],
                            in1=t1[:, m, :], op=mybir.AluOpType.add)
                        nc.scalar.activation(
                            tch[:, m, :], c_sb[:, m, :],
                            mybir.ActivationFunctionType.Tanh)
                        nc.vector.tensor_tensor(
                            out=hbf[:, m, :], in0=ifo[:, m, 600:900],
                            in1=tch[:, m, :], op=mybir.AluOpType.mult)
                        nc.vector.copy_predicated(
                            out=h_last[:, m, :],
                            mask=mask_sb[:, t, m:m + 1].to_broadcast([128, H]),
                            data=hbf[:, m, :])
                    for m in ms:
                        if t + 1 < steps[m]:
                            trp = ppool.tile([128, 3, 128], bf16, tag="tr")
                            for k, (d0, d1) in enumerate(H_SPLITS):
                                dk = d1 - d0
                                nc.tensor.transpose(
                                    out=trp[0:dk, k, :],
                                    in_=hbf[:, m, d0:d1],
                                    identity=identb[:, :])
                            # scaled fp8 convert during the psum->sbuf drain
                            nc.vector.tensor_scalar(
                                out=hT8[:, 0:2, m * 128:(m + 1) * 128],
                                in0=trp[:, 0:2, :], scalar1=S_H, scalar2=None,
                                op0=mybir.AluOpType.mult)
                            nc.vector.tensor_scalar(
                                out=hT8[0:44, 2, m * 128:(m + 1) * 128],
                                in0=trp[0:44, 2, :], scalar1=S_H, scalar2=None,
                                op0=mybir.AluOpType.mult)

                # ---------- phase 3: logits ----------
                trps = []
                for k in range(3):
                    trp = ppool.tile([128, 512], bf16, tag="gates")
                    trps.append(trp)
                for m in range(M):
                    for k, (d0, d1) in enumerate(H_SPLITS):
                        dk = d1 - d0
                        nc.tensor.transpose(
                            out=trps[k][0:dk, m * 128:(m + 1) * 128],
                            in_=h_last[:, m, d0:d1],
                            identity=identb[:, :])
                for k, (d0, d1) in enumerate(H_SPLITS):
                    dk = d1 - d0
                    nc.vector.tensor_copy(hlT[k][0:dk, :], trps[k][0:dk, :])

                lsb = wpool.tile([128, M, 2], f32, tag="lsb")
                for m in range(M):
                    lp = ppool.tile([128, 2], f32, tag="tr")
                    for k, (d0, d1) in enumerate(H_SPLITS):
                        nc.tensor.matmul(
                            lp[:, :],
                            lhsT=hlT[k][:, m * 128:(m + 1) * 128],
                            rhs=wc_sb[k][:, :],
                            start=(k == 0),
                            stop=(k == 2),
                        )
                    nc.vector.scalar_tensor_tensor(
                        out=lsb[:, m, :], in0=lp[:, :], scalar=1.0,
                        in1=clsb[:, :],
                        op0=mybir.AluOpType.mult, op1=mybir.AluOpType.add)
                nc.sync.dma_start(
                    out=out_d[:, :].rearrange("(m p) c -> p m c", p=128),
                    in_=lsb[:, :, :])

    nc.compile()
    return nc


_NC_CACHE = {}
LAST_RESULT = None


def _host_prep(inputs):
    cap = np.asarray(inputs["cap"]).astype(np.int64)
    cap_len = np.asarray(inputs["cap_len"]).astype(np.int64)
    embed_w = np.asarray(inputs["embed_w"], dtype=np.float32)
    W_ih = np.asarray(inputs["W_ih"], dtype=np.float32)
    W_hh = np.asarray(inputs["W_hh"], dtype=np.float32)
    b = (np.asarray(inputs["b_ih"], dtype=np.float32)
         + np.asarray(inputs["b_hh"], dtype=np.float32))
    cls_v = np.asarray(inputs["cls_v"], dtype=np.float32)
    cls_g = np.asarray(inputs["cls_g"], dtype=np.float32)
    cls_b = np.asarray(inputs["cls_b"], dtype=np.float32)

    # gate order [i f o g]
    perm = np.concatenate([np.arange(0, 300), np.arange(300, 600),
                           np.arange(900, 1200), np.arange(600, 900)])
    wih_aug = np.zeros((E + 1, GC), np.float32)
    wih_aug[:E] = W_ih[perm].T
    wih_aug[E] = b[perm]
    embT_aug = np.ones((E + 1, V), np.float32)
    embT_aug[:E] = embed_w.T

    Wp = W_hh[perm].T * S_W                          # [300, 1200], scaled
    waug2 = np.zeros((128, 4, GC), np.float32)
    waug2[:, 0, :] = Wp[0:128]
    waug2[:, 1, :] = Wp[128:256]
    waug2[0:44, 2, :] = Wp[256:300]

    Wc = cls_g * cls_v / np.linalg.norm(cls_v, axis=1, keepdims=True)  # [2, 300]

    # global sort by length; deal round-robin to cores
    order = np.argsort(cap_len, kind="stable")
    steps = []
    for m in range(M):
        mx = 0
        for c in range(NCORES):
            sel = order[c::NCORES]
            mx = max(mx, int(cap_len[sel[m * 128:(m + 1) * 128]].max()))
        steps.append(mx)
    steps = tuple(steps)

    # per-core used-vocab compaction
    used_l, inv_l = [], []
    for c in range(NCORES):
        sel = order[c::NCORES]
        used = np.unique(cap[sel])
        inv = np.zeros(V, np.int64)
        inv[used] = np.arange(len(used))
        used_l.append(used)
        inv_l.append(inv)
    upad = -(-max(len(u) for u in used_l) // 512) * 512

    shared = {
        "identb": np.eye(128, dtype=np.float32).astype(np_bf16),
        "ident8": np.broadcast_to(
            np.eye(128, dtype=np.float32)[:, None, :], (128, 2, 128)
        ).astype(np_fp8),
        "wih": wih_aug.astype(np_bf16),
        "waug2": waug2.astype(np_fp8),
        "wc": Wc.T.astype(np_bf16),
        "clsb": np.tile(cls_b.reshape(1, 2), (128, 1)).astype(np.float32),
    }
    in_maps = []
    for core in range(NCORES):
        sel = order[core::NCORES]
        capc = inv_l[core][cap[sel]]                   # [512, 32] remapped
        lenc = cap_len[sel]                            # [512]
        cols = np.zeros(upad, np.int64)
        cols[:len(used_l[core])] = used_l[core]
        embT_c = embT_aug[:, cols].astype(np_bf16)
        idx = np.ascontiguousarray(
            capc.reshape(M, 128, T).transpose(1, 2, 0)).astype(np.int32)
        lm = lenc.reshape(M, 128).T                    # [128, M]
        mask = (lm[:, None, :] - 1 == np.arange(T)[None, :, None]).astype(np.uint8)
        in_maps.append(dict(shared, idx=idx, embT=embT_c,
                            mask=np.ascontiguousarray(mask)))
    return in_maps, order, steps, upad


def kernel(**inputs) -> np.ndarray:
    global LAST_RESULT
    from concourse.bass_utils import run_bass_kernel_spmd
    in_maps, order, steps, upad = _host_prep(inputs)
    if (steps, upad) not in _NC_CACHE:
        _NC_CACHE[(steps, upad)] = build_bass(steps, upad)
    nc = _NC_CACHE[(steps, upad)]
    trace = bool(int(os.environ.get("KERNEL_TRACE", "0")))
    res = run_bass_kernel_spmd(nc, in_maps, core_ids=list(range(NCORES)), trace=trace)
    LAST_RESULT = res
    out = np.empty((B, 2), np.float32)
    for core in range(NCORES):
        out[order[core::NCORES]] = res.results[core]["out"].astype(np.float32)
    return out


# revision 28
# speedup vs baseline: 1.6432x; 1.0168x over previous
"""Trainium2 Bass kernel for nn_Discriminator_lstm (B=4096, T=32, E=H=300, VOCAB=10000).

Strategy (data-parallel over batch, 8 cores x 512 rows):
  Host: globally sort rows by cap_len, deal ranks round-robin to cores
        (every core gets the same length distribution), sorted ascending
        within each core.  m-tile m then has max length steps[m] (~8/16/24/32),
        and the recurrence runs only steps[m] steps for tile m.
  Phase 1 (per core): G = [embT;1] ^T @ [wih;b] in bf16 -> DRAM scratch
        [10000, 1200], scaled by S=512 (bias folded in via ones-row).
  Phase 2: per live (t, m):
        - indirect-DMA gather xg = G[cap[:, t]] (bf16) into SBUF
        - PE: inject xg into PSUM (identity matmul, bf16), then accumulate
          gates += hT8^T @ waug2 as TWO fp8e4m3 DoubleRow matmuls
          (k-slots [0:2] and [2:4]; h scaled by s_h=16, W_hh by s_w=32,
          psum scale S = s_h*s_w = 512)
        - ACT: sigmoid/tanh with scale=1/S -> bf16
        - DVE: c = f*c + i*g (f32); h_bf = o*tanh(c); masked h_last capture
        - Pool: h8 = (o*s_h)*tanh(c) -> fp8 for the next step's matmul
        - PE: transpose h8 (fp8, 1 cyc/row) -> one merged DVE copy into hT8
  Phase 3: logits = h_last @ Wc^T + cls_b in bf16.
"""

import os
import sys

import numpy as np

for _p in ("/opt/trn_rl_repo", "/root/.axon_site/_ro/trn_rl_repo"):
    if os.path.isdir(_p) and _p not in sys.path:
        sys.path.insert(0, _p)

import ml_dtypes

import concourse.bass as bass
import concourse.bacc as bacc
import concourse.mybir as mybir
import concourse.tile as tile

f32 = mybir.dt.float32
bf16 = mybir.dt.bfloat16
fp8 = mybir.dt.float8e4
i32 = mybir.dt.int32
u8 = mybir.dt.uint8

np_bf16 = ml_dtypes.bfloat16
np_fp8 = ml_dtypes.float8_e4m3

B, T, V, E, H = 4096, 32, 10000, 300, 300
NCORES = 8
BC = B // NCORES          # 512 batch rows per core
M = BC // 128             # 4 m-tiles
GC = 1200                 # 4*300 gate columns
CHUNKS = [(0, 512), (512, 1024), (1024, 1200)]
E_SPLITS = [(0, 128), (128, 256), (256, 301)]   # k-tiles of the [emb|1] contraction
H_SPLITS = [(0, 128), (128, 256), (256, 300)]   # h-dim splits for transposes/classifier
VTILES = (V + 127) // 128                        # 79

S_H = 16.0                # fp8 h scale
S_W = 32.0                # fp8 W_hh scale
S = S_H * S_W             # psum gate scale


def _raw(inst):
    return getattr(inst, "ins", inst)


def build_bass(steps, upad):
    nc = bacc.Bacc("TRN2", target_bir_lowering=False, debug=False, num_devices=NCORES)

    embT_d = nc.dram_tensor("embT", [E + 1, upad], bf16, kind="ExternalInput")
    wih_d = nc.dram_tensor("wih", [E + 1, GC], bf16, kind="ExternalInput")
    waug2_d = nc.dram_tensor("waug2", [128, 4, GC], fp8, kind="ExternalInput")
    wc_d = nc.dram_tensor("wc", [H, 2], bf16, kind="ExternalInput")
    clsb_d = nc.dram_tensor("clsb", [128, 2], f32, kind="ExternalInput")
    identb_d = nc.dram_tensor("identb", [128, 128], bf16, kind="ExternalInput")
    ident8_d = nc.dram_tensor("ident8", [128, 2, 128], fp8, kind="ExternalInput")
    idx_d = nc.dram_tensor("idx", [128, T, M], i32, kind="ExternalInput")
    mask_d = nc.dram_tensor("mask", [128, T, M], u8, kind="ExternalInput")
    G_d = nc.dram_tensor("G", [upad, GC], bf16, kind="Internal")
    out_d = nc.dram_tensor("out", [BC, 2], f32, kind="ExternalOutput")

    with tile.TileContext(nc, num_cores=NCORES) as tc:
        with (
            tc.tile_pool(name="const", bufs=1) as cpool,
            tc.tile_pool(name="state", bufs=1) as spool,
            tc.tile_pool(name="psum", bufs=2, space="PSUM") as ppool,
        ):
            # ---------- constants ----------
            identb = cpool.tile([128, 128], bf16, tag="identb")
            nc.sync.dma_start(out=identb[:, :], in_=identb_d[:, :])
            ident8 = cpool.tile([128, 2, 128], fp8, tag="ident8")
            nc.sync.dma_start(out=ident8[:, :, :], in_=ident8_d[:, :, :])
            waug2 = cpool.tile([128, 4, GC], fp8, tag="waug2")
            nc.sync.dma_start(out=waug2[:, :, :], in_=waug2_d[:, :, :])
            wc_sb = []
            for k, (d0, d1) in enumerate(H_SPLITS):
                t_ = cpool.tile([d1 - d0, 2], bf16, tag=f"wc{k}")
                nc.sync.dma_start(out=t_[:, :], in_=wc_d[d0:d1, :])
                wc_sb.append(t_)
            clsb = cpool.tile([128, 2], f32, tag="clsb")
            nc.sync.dma_start(out=clsb[:, :], in_=clsb_d[:, :])
            idx_sb = cpool.tile([128, T, M], i32, tag="idx")
            nc.sync.dma_start(out=idx_sb[:, :, :], in_=idx_d[:, :, :])
            mask_sb = cpool.tile([128, T, M], u8, tag="mask")
            nc.sync.dma_start(out=mask_sb[:, :, :], in_=mask_d[:, :, :])

            # ---------- state ----------
            # hT8 slots: [0]=h rows 0:128, [1]=128:256, [2]=256:300 (+garbage,
            # nulled by zero rows in waug2), [3]=zeros in waug2 -> don't care.
            hT8 = spool.tile([128, 4, BC], fp8, tag="hT8")
            nc.gpsimd.memset(hT8[:, :, :], 0.0)
            c_sb = spool.tile([128, M, H], bf16, tag="c")
            nc.gpsimd.memset(c_sb[:, :, :], 0.0)
            h_last = spool.tile([128, M, H], bf16, tag="hlast")
            hlT = []
            for k, (d0, d1) in enumerate(H_SPLITS):
                t_ = spool.tile([d1 - d0, BC], bf16, tag=f"hlT{k}")
                hlT.append(t_)

            # ---------- phase 1: G = [embT;1]^T @ [wih;b], scaled by S ----------
            g_stores = []
            with tc.tile_pool(name="gphase", bufs=1) as gpool, \
                 tc.tile_pool(name="gsbp", bufs=12) as gsbp:
                wih_sb = []
                for k, (d0, d1) in enumerate(E_SPLITS):
                    t_ = gpool.tile([d1 - d0, GC], bf16, tag=f"wih{k}")
                    nc.sync.dma_start(out=t_[:, :], in_=wih_d[d0:d1, :])
                    wih_sb.append(t_)
                embT_sb = []
                for k, (d0, d1) in enumerate(E_SPLITS):
                    t_ = gpool.tile([d1 - d0, upad], bf16, tag=f"emb{k}")
                    embT_sb.append(t_)
                # column-major load order so early vtiles unblock quickly
                NQ = 8
                for q in range(NQ):
                    q0, q1 = q * (upad // NQ), (q + 1) * (upad // NQ)
                    for k, (d0, d1) in enumerate(E_SPLITS):
                        nc.sync.dma_start(
                            out=embT_sb[k][:, q0:q1],
                            in_=embT_d[d0:d1, q0:q1])

                for v in range(upad // 128):
                    rows = 128
                    gps = ppool.tile([128, GC], f32, tag="gates")
                    for k, (d0, d1) in enumerate(E_SPLITS):
                        for (c0, c1) in CHUNKS:
                            nc.tensor.matmul(
                                gps[0:rows, c0:c1],
                                lhsT=embT_sb[k][:, v * 128:v * 128 + rows],
                                rhs=wih_sb[k][:, c0:c1],
                                start=(k == 0),
                                stop=(k == 2),
                            )
                    gsb = gsbp.tile([128, GC], bf16, tag="gsb")
                    nc.scalar.activation(
                        gsb[0:rows, 0:600], gps[0:rows, 0:600],
                        mybir.ActivationFunctionType.Copy, scale=S)
                    nc.vector.tensor_scalar(
                        out=gsb[0:rows, 600:1200], in0=gps[0:rows, 600:1200],
                        scalar1=S, scalar2=None, op0=mybir.AluOpType.mult)
                    st = nc.sync.dma_start(
                        out=G_d[v * 128:v * 128 + rows, :], in_=gsb[0:rows, :]
                    )
                    g_stores.append(st)

            g_done = nc.gpsimd.nop()
            for st in g_stores:
                tile.add_dep_helper(_raw(g_done), _raw(st), reason="G stored")

            # ---------- phase 2: recurrence ----------
            with tc.tile_pool(name="work", bufs=2) as wpool:
                for t in range(T):
                    ms = [m for m in range(M) if steps[m] > t]
                    xg = wpool.tile([128, M, GC], bf16, tag="xg")
                    for m in ms:
                        gather = nc.gpsimd.indirect_dma_start(
                            out=xg[:, m, :],
                            out_offset=None,
                            in_=G_d[:, :],
                            in_offset=bass.IndirectOffsetOnAxis(
                                ap=idx_sb[:, t, m:m + 1], axis=0),
                        )
                        tile.add_dep_helper(_raw(gather), _raw(g_done),
                                            reason="gather after G")

                    ifo = wpool.tile([128, M, 900], bf16, tag="ifo")
                    gt = wpool.tile([128, M, 300], bf16, tag="gt")
                    tch = wpool.tile([128, M, H], bf16, tag="tch")
                    hbf = wpool.tile([128, M, H], bf16, tag="hbf")
                    t1 = wpool.tile([128, M, H], bf16, tag="t1")
                    for m in ms:
                        gps = ppool.tile([128, GC], f32, tag="gates")
                        # inject xg (clears + seeds accumulation per chunk)
                        for (c0, c1) in CHUNKS:
                            nc.tensor.matmul(
                                gps[:, c0:c1],
                                lhsT=identb[:, :],
                                rhs=xg[:, m, c0:c1],
                                start=True,
                                stop=(t == 0),
                            )
                        if t > 0:
                            # gates += hT8^T @ waug2, fp8 DoubleRow (2 k-slot pairs)
                            for j, (s0, s1) in enumerate(((0, 2), (2, 4))):
                                for (c0, c1) in CHUNKS:
                                    nc.tensor.matmul(
                                        gps[:, c0:c1],
                                        lhsT=hT8[:, s0:s1, m * 128:(m + 1) * 128],
                                        rhs=waug2[:, s0:s1, c0:c1],
                                        start=False,
                                        stop=(j == 1),
                                        perf_mode=mybir.MatmulPerfMode.DoubleRow,
                                    )
                        nc.scalar.activation(
                            ifo[:, m, 0:600], gps[:, 0:600],
                            mybir.ActivationFunctionType.Sigmoid, scale=1.0 / S)
                        nc.scalar.activation(
                            gt[:, m, :], gps[:, 900:1200],
                            mybir.ActivationFunctionType.Tanh, scale=1.0 / S)
                        nc.scalar.activation(
                            ifo[:, m, 600:900], gps[:, 600:900],
                            mybir.ActivationFunctionType.Sigmoid, scale=1.0 / S)
                        # c = f*c + i*g ; h = o*tanh(c)
                        nc.vector.tensor_tensor(
                            out=t1[:, m, :], in0=ifo[:, m, 300:600],
                            in1=c_sb[:, m, :], op=mybir.AluOpType.mult)
                        nc.vector.tensor_tensor(
                            out=c_sb[:, m, :], in0=ifo[:, m, 0:300],
                            in1=gt[:, m, :], op=mybir.AluOpType.mult)
                        nc.vector.tensor_tensor(
                            out=c_sb[:, m, :], in0=c_sb[:, m, :],
                            in1=t1[:, m, :], op=mybir.AluOpType.add)
                        nc.scalar.activation(
                            tch[:, m, :], c_sb[:, m, :],
                            mybir.ActivationFunctionType.Tanh)
                        nc.vector.tensor_tensor(
                            out=hbf[:, m, :], in0=ifo[:, m, 600:900],
                            in1=tch[:, m, :], op=mybir.AluOpType.mult)
                        nc.vector.copy_predicated(
                            out=h_last[:, m, :],
                            mask=mask_sb[:, t, m:m + 1].to_broadcast([128, H]),
                            data=hbf[:, m, :])
                    for m in ms:
                        if t + 1 < steps[m]:
                            trp = ppool.tile([128, 3, 128], bf16, tag="tr")
                            for k, (d0, d1) in enumerate(H_SPLITS):
                                dk = d1 - d0
                                nc.tensor.transpose(
                                    out=trp[0:dk, k, :],
                                    in_=hbf[:, m, d0:d1],
                                    identity=identb[:, :])
                            # scaled fp8 convert during the psum->sbuf drain
                            nc.vector.tensor_scalar(
                                out=hT8[:, 0:2, m * 128:(m + 1) * 128],
                                in0=trp[:, 0:2, :], scalar1=S_H, scalar2=None,
                                op0=mybir.AluOpType.mult)
                            nc.vector.tensor_scalar(
                                out=hT8[0:44, 2, m * 128:(m + 1) * 128],
                                in0=trp[0:44, 2, :], scalar1=S_H, scalar2=None,
                                op0=mybir.AluOpType.mult)

                # ---------- phase 3: logits ----------
                trps = []
                for k in range(3):
                    trp = ppool.tile([128, 512], bf16, tag="gates")
                    trps.append(trp)
                for m in range(M):
                    for k, (d0, d1) in enumerate(H_SPLITS):
                        dk = d1 - d0
                        nc.tensor.transpose(
                            out=trps[k][0:dk, m * 128:(m + 1) * 128],
                            in_=h_last[:, m, d0:d1],
                            identity=identb[:, :])
                for k, (d0, d1) in enumerate(H_SPLITS):
                    dk = d1 - d0
                    nc.vector.tensor_copy(hlT[k][0:dk, :], trps[k][0:dk, :])

                lsb = wpool.tile([128, M, 2], f32, tag="lsb")
                for m in range(M):
                    lp = ppool.tile([128, 2], f32, tag="tr")
                    for k, (d0, d1) in enumerate(H_SPLITS):
                        nc.tensor.matmul(
                            lp[:, :],
                            lhsT=hlT[k][:, m * 128:(m + 1) * 128],
                            rhs=wc_sb[k][:, :],
                            start=(k == 0),
                            stop=(k == 2),
                        )
                    nc.vector.scalar_tensor_tensor(
                        out=lsb[:, m, :], in0=lp[:, :], scalar=1.0,
                        in1=clsb[:, :],
                        op0=mybir.AluOpType.mult, op1=mybir.AluOpType.add)
                nc.sync.dma_start(
                    out=out_d[:, :].rearrange("(m p) c -> p m c", p=128),
                    in_=lsb[:, :, :])

    nc.compile()
    return nc


_NC_CACHE = {}
LAST_RESULT = None


def _host_prep(inputs):
    cap = np.asarray(inputs["cap"]).astype(np.int64)
    cap_len = np.asarray(inputs["cap_len"]).astype(np.int64)
    embed_w = np.asarray(inputs["embed_w"], dtype=np.float32)
    W_ih = np.asarray(inputs["W_ih"], dtype=np.float32)
    W_hh = np.asarray(inputs["W_hh"], dtype=np.float32)
    b = (np.asarray(inputs["b_ih"], dtype=np.float32)
         + np.asarray(inputs["b_hh"], dtype=np.float32))
    cls_v = np.asarray(inputs["cls_v"], dtype=np.float32)
    cls_g = np.asarray(inputs["cls_g"], dtype=np.float32)
    cls_b = np.asarray(inputs["cls_b"], dtype=np.float32)

    # gate order [i f o g]
    perm = np.concatenate([np.arange(0, 300), np.arange(300, 600),
                           np.arange(900, 1200), np.arange(600, 900)])
    wih_aug = np.zeros((E + 1, GC), np.float32)
    wih_aug[:E] = W_ih[perm].T
    wih_aug[E] = b[perm]
    embT_aug = np.ones((E + 1, V), np.float32)
    embT_aug[:E] = embed_w.T

    Wp = W_hh[perm].T * S_W                          # [300, 1200], scaled
    waug2 = np.zeros((128, 4, GC), np.float32)
    waug2[:, 0, :] = Wp[0:128]
    waug2[:, 1, :] = Wp[128:256]
    waug2[0:44, 2, :] = Wp[256:300]

    Wc = cls_g * cls_v / np.linalg.norm(cls_v, axis=1, keepdims=True)  # [2, 300]

    # global sort by length; deal round-robin to cores
    order = np.argsort(cap_len, kind="stable")
    steps = []
    for m in range(M):
        mx = 0
        for c in range(NCORES):
            sel = order[c::NCORES]
            mx = max(mx, int(cap_len[sel[m * 128:(m + 1) * 128]].max()))
        steps.append(mx)
    steps = tuple(steps)

    # per-core used-vocab compaction
    used_l, inv_l = [], []
    for c in range(NCORES):
        sel = order[c::NCORES]
        used = np.unique(cap[sel])
        inv = np.zeros(V, np.int64)
        inv[used] = np.arange(len(used))
        used_l.append(used)
        inv_l.append(inv)
    upad = -(-max(len(u) for u in used_l) // 512) * 512

    shared = {
        "identb": np.eye(128, dtype=np.float32).astype(np_bf16),
        "ident8": np.broadcast_to(
            np.eye(128, dtype=np.float32)[:, None, :], (128, 2, 128)
        ).astype(np_fp8),
        "wih": wih_aug.astype(np_bf16),
        "waug2": waug2.astype(np_fp8),
        "wc": Wc.T.astype(np_bf16),
        "clsb": np.tile(cls_b.reshape(1, 2), (128, 1)).astype(np.float32),
    }
    in_maps = []
    for core in range(NCORES):
        sel = order[core::NCORES]
        capc = inv_l[core][cap[sel]]                   # [512, 32] remapped
        lenc = cap_len[sel]                            # [512]
        cols = np.zeros(upad, np.int64)
        cols[:len(used_l[core])] = used_l[core]
        embT_c = embT_aug[:, cols].astype(np_bf16)
        idx = np.ascontiguousarray(
            capc.reshape(M, 128, T).transpose(1, 2, 0)).astype(np.int32)
        lm = lenc.reshape(M, 128).T                    # [128, M]
        mask = (lm[:, None, :] - 1 == np.arange(T)[None, :, None]).astype(np.uint8)
        in_maps.append(dict(shared, idx=idx, embT=embT_c,
                            mask=np.ascontiguousarray(mask)))
    return in_maps, order, steps, upad


def kernel(**inputs) -> np.ndarray:
    global LAST_RESULT
    from concourse.bass_utils import run_bass_kernel_spmd
    in_maps, order, steps, upad = _host_prep(inputs)
    if (steps, upad) not in _NC_CACHE:
        _NC_CACHE[(steps, upad)] = build_bass(steps, upad)
    nc = _NC_CACHE[(steps, upad)]
    trace = bool(int(os.environ.get("KERNEL_TRACE", "0")))
    res = run_bass_kernel_spmd(nc, in_maps, core_ids=list(range(NCORES)), trace=trace)
    LAST_RESULT = res
    out = np.empty((B, 2), np.float32)
    for core in range(NCORES):
        out[order[core::NCORES]] = res.results[core]["out"].astype(np.float32)
    return out
